# revision 1
# baseline (speedup 1.0000x reference)
"""Involution (B=4, C=256, H=W=56, K=7, G=16, reduction=4) on 8 trn2 NeuronCores.

Sharding: 8 shards = (batch b in 0..3) x (h-half in 0..1); each core computes
its [256, 28, 56] output slab from a [256, 34, 62] zero-padded input slab.

Per-core pipeline:
  1. matmul1 (PE, bf16): t = relu(bn(W1 @ x)) with BN folded into W1/b1 on host
  2. per-tap "broadcast matmul" (PE, bf16): for each of the 49 kernel taps,
     lhsT = W2bc[k] (a host-precomputed [65,128] slice of W2 whose columns are
     replicated 16x across each group's channels, row 64 carrying b2) so the
     matmul directly yields the per-pixel kernel values broadcast over the 16
     channels of each group: wbc[k][c, p] in PSUM.
  3. involution accumulation (DVE): acc[c,p] += x[c, p+delta_k] * wbc[k][c,p]
"""

import numpy as np
import ml_dtypes
from contextlib import ExitStack

import concourse.bass as bass
import concourse.bacc as bacc
import concourse.tile as tile
from concourse import mybir
from concourse.bass_utils import run_bass_kernel_spmd

BF16 = ml_dtypes.bfloat16

B, C, H, W = 4, 256, 56, 56
KK, G, PAD = 7, 16, 3
Cr, Cg = 64, 16
EPS = 1e-5
HH = H // 2              # 28 rows per h-half shard
PH, PW = HH + 2 * PAD, W + 2 * PAD   # 34, 62 padded slab dims
NPIX = HH * W            # 1568 output pixels per shard
NCORES = 8

_CACHE = {}

# set by test.py to collect a hardware profile
TRACE = False
LAST_RESULT = None


def _build_nc():
    nc = bacc.Bacc("TRN2", target_bir_lowering=False, debug=False,
                   num_devices=NCORES)

    f32 = mybir.dt.float32
    bf16 = mybir.dt.bfloat16

    x_d = nc.declare_dram_parameter("x", [C, PH, PW], f32, isOutput=False)
    w1t_d = nc.declare_dram_parameter("w1t", [2, 128, Cr], bf16, isOutput=False)
    b1p_d = nc.declare_dram_parameter("b1p", [Cr, 1], f32, isOutput=False)
    w2bc_d = nc.declare_dram_parameter("w2bc", [Cr + 1, 49, 2, 128], bf16,
                                       isOutput=False)
    out_d = nc.declare_dram_parameter("out", [C, HH, W], f32, isOutput=True)

    with tile.TileContext(nc) as tc, ExitStack() as ctx:
        const = ctx.enter_context(tc.tile_pool(name="const", bufs=1))
        xpool = ctx.enter_context(tc.tile_pool(name="x", bufs=1))
        tpool = ctx.enter_context(tc.tile_pool(name="t", bufs=1))

        # constants
        w1t_sb = const.tile([128, 2, Cr], bf16)
        for ch in range(2):
            nc.sync.dma_start(w1t_sb[:, ch, :], w1t_d[ch])
        b1p_sb = const.tile([Cr, 1], f32)
        nc.sync.dma_start(b1p_sb[:], b1p_d[:])
        w2bc_sb = const.tile([Cr + 1, 49, 2, 128], bf16)
        nc.sync.dma_start(w2bc_sb[:], w2bc_d[:])

        # input slabs (already zero-padded on host), one per channel half.
        # xb_e is the bf16 cast; xb_o is the same shifted left one column so
        # odd-j tap windows stay 4B-aligned (keeps DVE tensor_tensor in 2x).
        x_bf16 = []
        x_bf16_odd = []
        for ch in range(2):
            xf = xpool.tile([128, PH, PW], f32, tag=f"xf{ch}")
            nc.sync.dma_start(xf[:], x_d[ch * 128:(ch + 1) * 128])
            xb = xpool.tile([128, PH, PW], bf16, tag=f"xb{ch}")
            nc.vector.tensor_copy(xb[:], xf[:])
            x_bf16.append(xb)
            xo = xpool.tile([128, PH, PW - 2], bf16, tag=f"xo{ch}")
            nc.vector.tensor_copy(xo[:], xf[:, :, 1:PW - 1])
            x_bf16_odd.append(xo)

        # ---- stage 1: t_ext = [relu(W1p @ x + b1p); ones] in bf16 ----
        t_ext = tpool.tile([Cr + 1, NPIX], bf16)
        nc.vector.memset(t_ext[Cr:Cr + 1, :], 1.0)
        with tc.tile_pool(name="psum_t", bufs=2,
                          space=bass.MemorySpace.PSUM) as psum_t:
            NROW_CHUNK = 7          # 7 rows x 56 cols = 392 <= 512 (one bank)
            for q in range(HH // NROW_CHUNK):
                pt = psum_t.tile([Cr, NROW_CHUNK * W], f32)
                for ch in range(2):
                    rhs = x_bf16[ch][:, PAD + q * NROW_CHUNK:
                                     PAD + (q + 1) * NROW_CHUNK, PAD:PAD + W]
                    nc.tensor.matmul(pt[:], w1t_sb[:, ch, :], rhs,
                                     start=(ch == 0), stop=(ch == 1))
                nc.scalar.activation(
                    t_ext[0:Cr, q * NROW_CHUNK * W:(q + 1) * NROW_CHUNK * W],
                    pt[:], mybir.ActivationFunctionType.Relu,
                    bias=b1p_sb[:], scale=1.0)

        # ---- stage 2+3: per-tap broadcast matmul + multiply-accumulate ----
        accp = ctx.enter_context(tc.tile_pool(name="acc", bufs=1))
        wbcp = ctx.enter_context(tc.tile_pool(name="wbc", bufs=3))
        outp = ctx.enter_context(tc.tile_pool(name="outp", bufs=2))
        psum_w = ctx.enter_context(
            tc.tile_pool(name="psum_w", bufs=2, space=bass.MemorySpace.PSUM))

        NACC = 4   # parallel bf16 accumulators (keeps rounding error down)
        MM_CHUNKS = [(0, 512), (512, 512), (1024, 512), (1536, 32)]
        for ch in range(2):
            accs = [accp.tile([128, HH, W], bf16, tag=f"acc{ch}_{m}",
                              name=f"acc{ch}_{m}")
                    for m in range(NACC)]
            for k in range(49):
                pw = psum_w.tile([128, NPIX], f32, tag="pw")
                for (o, n) in MM_CHUNKS:
                    nc.tensor.matmul(pw[:, o:o + n], w2bc_sb[:, k, ch, :],
                                     t_ext[:, o:o + n], start=True, stop=True)
                wbc = wbcp.tile([128, HH, W], bf16, tag="wbc")
                nc.scalar.copy(wbc[:].rearrange("p h w -> p (h w)"), pw[:])
                i, j = k // KK, k % KK
                if j % 2 == 0:
                    xwin = x_bf16[ch][:, i:i + HH, j:j + W]
                else:
                    xwin = x_bf16_odd[ch][:, i:i + HH, j - 1:j - 1 + W]
                m = k % NACC
                if k < NACC:
                    nc.vector.tensor_mul(accs[k][:], xwin, wbc[:])
                else:
                    nc.vector.tensor_mul(wbc[:], xwin, wbc[:])
                    if m == 3 or (m == 1 and k >= 21):
                        nc.gpsimd.tensor_add(accs[m][:], accs[m][:], wbc[:])
                    else:
                        nc.vector.tensor_add(accs[m][:], accs[m][:], wbc[:])
            s0 = wbcp.tile([128, HH, W], bf16, tag="wbc")
            nc.vector.tensor_add(s0[:], accs[0][:], accs[1][:])
            s1 = wbcp.tile([128, HH, W], bf16, tag="wbc")
            nc.vector.tensor_add(s1[:], accs[2][:], accs[3][:])
            of = outp.tile([128, HH, W], f32, tag="of")
            nc.vector.tensor_add(of[:], s0[:], s1[:])
            nc.sync.dma_start(out_d[ch * 128:(ch + 1) * 128], of[:])

    nc.compile()
    return nc


def _prep_host_inputs(inputs, W1, b1, gamma, beta, mean, var, W2, b2):
    """Fold BN into W1/b1; build per-tap channel-broadcast W2 slices."""
    scale = gamma / np.sqrt(var + EPS)
    shift = beta - mean * scale
    W1p = W1 * scale[:, None]
    b1p = (b1 * scale + shift).astype(np.float32).reshape(Cr, 1)
    w1t = np.ascontiguousarray(W1p.T.reshape(2, 128, Cr)).astype(BF16)

    O = ((np.arange(2)[None, :, None] * 8
          + np.arange(128)[None, None, :] // 16) * 49
         + np.arange(49)[:, None, None])          # [49, 2, 128]
    w2bc = np.zeros((Cr + 1, 49, 2, 128), np.float32)
    w2bc[0:Cr] = np.transpose(W2[O, :], (3, 0, 1, 2))
    w2bc[Cr] = b2[O]
    w2bc = w2bc.astype(BF16)

    # per-core padded input slabs
    xs = []
    for core in range(NCORES):
        b, hf = core // 2, core % 2
        slab = np.zeros((C, PH, PW), np.float32)
        r0 = hf * HH - PAD
        r1 = r0 + PH
        v0, v1 = max(r0, 0), min(r1, H)
        slab[:, v0 - r0:v1 - r0, PAD:PAD + W] = inputs[b, :, v0:v1, :]
        xs.append(slab)
    return xs, w1t, b1p, w2bc


def kernel(inputs, W1, b1, gamma, beta, mean, var, W2, b2):
    global LAST_RESULT
    inputs = np.asarray(inputs, np.float32)
    if "nc" not in _CACHE:
        _CACHE["nc"] = _build_nc()
    nc = _CACHE["nc"]

    xs, w1t, b1p, w2bc = _prep_host_inputs(
        inputs, np.asarray(W1, np.float32), np.asarray(b1, np.float32),
        np.asarray(gamma, np.float32), np.asarray(beta, np.float32),
        np.asarray(mean, np.float32), np.asarray(var, np.float32),
        np.asarray(W2, np.float32), np.asarray(b2, np.float32))

    in_maps = [{"x": xs[core], "w1t": w1t, "b1p": b1p, "w2bc": w2bc}
               for core in range(NCORES)]
    res = run_bass_kernel_spmd(nc, in_maps, list(range(NCORES)), trace=TRACE)
    LAST_RESULT = res

    out = np.empty((B, C, H, W), np.float32)
    for core in range(NCORES):
        b, hf = core // 2, core % 2
        out[b, :, hf * HH:(hf + 1) * HH, :] = res.results[core]["out"]
    return out



# revision 2
# speedup vs baseline: 1.2104x; 1.2104x over previous
"""Involution (B=4, C=256, H=W=56, K=7, G=16, reduction=4) on 8 trn2 NeuronCores.

v2: pixel-major involution with zero-materialization weight broadcast.

Sharding: 8 shards = (batch b in 0..3) x (h-half hf in 0..1); each core
computes a [256, 28, 56] output slab.

Per-core layout (all bf16 in SBUF):
  - Pixel-major transposed input xT[124, (r:20, cg:16, g:16)]: partition
    p = 62*ck + pw where ck splits the 28 output rows into 2x14 and pw is
    the padded column (0..61); r is the padded row within the ck chunk
    (14+2*3=20); channel c = g*16+cg stored g-innermost so that per-group
    weights broadcast via a stride-0 middle AP dim.
  - Stage 1 (PE+ACT): t_ext[65, (rh:14, ckpw:118)] = [relu(W1p@x+b1p); 1]
    from channel-major x_cm; BN folded into W1p/b1p on host.
  - Stage 2 (PE+ACT): per rh: psum[118, 784] = t_ext[:,rh,:]^T @ W2e
    (cols n = gh*392 + k*8 + g8), one ACT copy -> wT[124,(rh,k,g)] bf16.
  - Stage 3 (DVE+GPSIMD): per tap k=(i,j), ONE op over the whole slab:
      acc += xT[j:j+118, i*256 : i*256+3584] * wT[0:118, :, k, :]-bcast
    The weight read uses AP [(rh,784),(cg,0),(g,1)] - stride-0 broadcast,
    verified to run in DVE 2x mode. ~11 taps run on gpsimd instead.
  - Merge accumulators -> f32, DMA out; host un-permutes.
"""

import numpy as np
import ml_dtypes
from contextlib import ExitStack

import concourse.bass as bass
import concourse.bacc as bacc
import concourse.tile as tile
from concourse import mybir
from concourse.bass_utils import run_bass_kernel_spmd

BF16 = ml_dtypes.bfloat16

B, C, H, W = 4, 256, 56, 56
KK, G, PAD = 7, 16, 3
Cr, Cg = 64, 16
EPS = 1e-5
HH = H // 2              # 28 rows per h-half shard
HQ = HH // 2             # 14 rows per sub-chunk (ck)
PW = W + 2 * PAD         # 62 padded width
PR = HQ + 2 * PAD        # 20 padded rows per sub-chunk
NCOL = PW + W            # 118 = ck0 cols 0..61 + ck1 cols 0..55
NPIX = HQ * NCOL         # 1652 stage-1/2 pixel columns
FREE3 = HQ * 256         # 3584 = stage-3 free size
NCORES = 8

# GPSIMD shares SBUF ports with the DVE: running Pool tensor_tensor ops
# concurrently with DVE taps collapses DVE throughput ~4.3x (measured), so
# stage 3 runs entirely on the DVE.
N_DVE_ACC = 4

_CACHE = {}
TRACE = False
LAST_RESULT = None


def _build_nc():
    nc = bacc.Bacc("TRN2", target_bir_lowering=False, debug=False,
                   num_devices=NCORES)

    f32 = mybir.dt.float32
    bf16 = mybir.dt.bfloat16

    xcm_d = nc.declare_dram_parameter("xcm", [2, 128, NPIX], bf16,
                                      isOutput=False)
    # 134 rows: 124 (ck,pw) rows + 10 zero rows so each shifted load
    # xTs[j] = xt[j : j+128] stays in range for j <= 6
    xt_d = nc.declare_dram_parameter("xt", [134, PR * 256], bf16,
                                     isOutput=False)
    w1t_d = nc.declare_dram_parameter("w1t", [2, 128, Cr], bf16,
                                      isOutput=False)
    b1p_d = nc.declare_dram_parameter("b1p", [Cr, 1], f32, isOutput=False)
    w2e_d = nc.declare_dram_parameter("w2e", [Cr + 1, 784], bf16,
                                      isOutput=False)
    out_d = nc.declare_dram_parameter("out", [118, FREE3], bf16,
                                      isOutput=True)

    with tile.TileContext(nc) as tc, ExitStack() as ctx:
        const = ctx.enter_context(tc.tile_pool(name="const", bufs=1))
        xpool = ctx.enter_context(tc.tile_pool(name="x", bufs=1))
        tpool = ctx.enter_context(tc.tile_pool(name="t", bufs=1))
        wpool = ctx.enter_context(tc.tile_pool(name="w", bufs=1))

        # DMA queues run ~50 GB/s each (descriptor-rate-bound), so the
        # critical x_cm load is split across both HWDGE queues first, the
        # small constants follow, and the 7 big xTs loads round-robin over
        # sync / scalar / gpsimd(SWDGE) to overlap with the j-major taps.
        x_cm = xpool.tile([128, 2, NPIX], bf16)
        nc.sync.dma_start(x_cm[:, 0, :], xcm_d[0])
        nc.scalar.dma_start(x_cm[:, 1, :], xcm_d[1])

        w1t_sb = const.tile([128, 2, Cr], bf16)
        for ch in range(2):
            nc.sync.dma_start(w1t_sb[:, ch, :], w1t_d[ch])
        b1p_sb = const.tile([Cr, 1], f32)
        nc.sync.dma_start(b1p_sb[:], b1p_d[:])
        w2e_sb = const.tile([Cr + 1, 784], bf16)
        nc.sync.dma_start(w2e_sb[:], w2e_d[:])

        xTs = []
        dma_eng = [nc.gpsimd, nc.scalar, nc.sync]
        for j in range(KK):
            xtj = xpool.tile([128, PR * 256], bf16, name=f"xTs{j}")
            dma_eng[j % len(dma_eng)].dma_start(xtj[:], xt_d[j:j + 128])
            xTs.append(xtj)

        # ---- stage 1: t_ext = [relu(W1p @ x + b1p); ones] ----
        t_ext = tpool.tile([Cr + 1, HQ, NCOL], bf16)
        nc.vector.memset(t_ext[Cr:Cr + 1, :, :], 1.0)
        t2 = t_ext[:].rearrange("p a b -> p (a b)")
        with tc.tile_pool(name="psum_t", bufs=2,
                          space=bass.MemorySpace.PSUM) as psum_t:
            NCH = 413  # 4 chunks of 413 = 1652 (<= 512 per psum bank)
            for q in range(4):
                pt = psum_t.tile([Cr, NCH], f32)
                for ch in range(2):
                    nc.tensor.matmul(pt[:], w1t_sb[:, ch, :],
                                     x_cm[:, ch, q * NCH:(q + 1) * NCH],
                                     start=(ch == 0), stop=(ch == 1))
                nc.scalar.activation(
                    t2[0:Cr, q * NCH:(q + 1) * NCH],
                    pt[:], mybir.ActivationFunctionType.Relu,
                    bias=b1p_sb[:], scale=1.0)

        # ---- stage 2: per-rh weight matmul -> wT[(rh, k, g)] ----
        # psum->sbuf copies alternate between ACT and the (still idle) DVE
        # so the copy wall halves.
        wT = wpool.tile([124, HQ, 49, G], bf16)
        with tc.tile_pool(name="psum_w", bufs=4,
                          space=bass.MemorySpace.PSUM) as psum_w:
            for rh in range(HQ):
                # [118, 2, 512]: each gh half starts on a psum bank boundary
                pw = psum_w.tile([NCOL, 2, 512], f32)
                for gh in range(2):
                    nc.tensor.matmul(pw[:, gh, 0:392],
                                     t_ext[:, rh, :],
                                     w2e_sb[:, gh * 392:(gh + 1) * 392],
                                     start=True, stop=True)
                # psum col (gh, k*8+g8) -> wT[p, rh, k, gh*8+g8]
                dst = wT[0:NCOL, rh, :, :].rearrange(
                    "p k (gh g8) -> p k gh g8", gh=2)
                src = pw[:, :, 0:392].rearrange(
                    "p gh (k g8) -> p k gh g8", k=49)
                if rh % 2 == 0:
                    nc.scalar.copy(dst, src)
                else:
                    nc.vector.tensor_copy(dst, src)

        # ---- stage 3: involution taps ----
        accp = ctx.enter_context(tc.tile_pool(name="acc", bufs=1))
        prodp = ctx.enter_context(tc.tile_pool(name="prod", bufs=6))

        daccs = [accp.tile([124, FREE3], bf16, name=f"dacc{m}")
                 for m in range(N_DVE_ACC)]

        nd = 0
        # j-major tap order: tap (i, j) consumes xTs[j]; the shift DMAs for
        # larger j complete while earlier-j taps run.
        for j in range(KK):
          for i in range(KK):
            k = i * KK + j
            src = xTs[j][0:118, i * 256:i * 256 + FREE3]
            wsl = wT[0:118, :, k, :].unsqueeze(2).broadcast_to(
                [118, HQ, Cg, G])
            src4 = src.rearrange("p (rh c) -> p rh c", rh=HQ).rearrange(
                "p rh (cg g) -> p rh cg g", g=G)
            m = nd % N_DVE_ACC
            ddst = daccs[m][0:118, :].rearrange(
                "p (rh cg g) -> p rh cg g", rh=HQ, g=G)
            if nd < N_DVE_ACC:
                nc.vector.tensor_mul(ddst, src4, wsl)
            else:
                dp = prodp.tile([124, FREE3], bf16, tag="dp")
                dp4 = dp[0:118, :].rearrange(
                    "p (rh cg g) -> p rh cg g", rh=HQ, g=G)
                nc.vector.tensor_mul(dp4, src4, wsl)
                nc.vector.tensor_add(daccs[m][0:118, :],
                                     daccs[m][0:118, :], dp[0:118, :])
            nd += 1

        # ---- merge (in-place, bf16; host converts to f32) + output ----
        # output DMA cost is per-partition-descriptor bound: split by
        # PARTITION range across the three queues so they run in parallel.
        nc.vector.tensor_add(daccs[0][0:118], daccs[0][0:118],
                             daccs[1][0:118])
        nc.vector.tensor_add(daccs[2][0:118], daccs[2][0:118],
                             daccs[3][0:118])
        nc.vector.tensor_add(daccs[0][0:118], daccs[0][0:118],
                             daccs[2][0:118])
        nc.sync.dma_start(out_d[0:40], daccs[0][0:40, :])
        nc.scalar.dma_start(out_d[40:79], daccs[0][40:79, :])
        nc.gpsimd.dma_start(out_d[79:118], daccs[0][79:118, :])

    nc.compile()
    return nc


def _prep_host_inputs(inputs, W1, b1, gamma, beta, mean, var, W2, b2):
    """Fold BN into W1/b1; build per-core transposed layouts."""
    scale = gamma / np.sqrt(var + EPS)
    shift = beta - mean * scale
    W1p = W1 * scale[:, None]
    b1p = (b1 * scale + shift).astype(np.float32).reshape(Cr, 1)
    w1t = np.ascontiguousarray(W1p.T.reshape(2, 128, Cr)).astype(BF16)

    # W2e [65, 784]: row 64 = b2; col n = gh*392 + k*8 + g8 for g = gh*8+g8
    w2full = np.concatenate([W2, b2[:, None]], axis=1)  # [784, 65]
    w2g = w2full.reshape(G, 49, Cr + 1)                 # [g, k, cr+1]
    w2e = w2g.reshape(2, 8, 49, Cr + 1).transpose(3, 0, 2, 1)  # [65,2,49,8]
    w2e = np.ascontiguousarray(w2e.reshape(Cr + 1, 784)).astype(BF16)

    xcms, xts = [], []
    for core in range(NCORES):
        b, hf = core // 2, core % 2
        slab = np.zeros((C, HH + 2 * PAD, PW), np.float32)
        r0 = hf * HH - PAD
        r1 = r0 + HH + 2 * PAD
        v0, v1 = max(r0, 0), min(r1, H)
        slab[:, v0 - r0:v1 - r0, PAD:PAD + W] = inputs[b, :, v0:v1, :]
        slab = slab.astype(BF16)  # [256, 34, 62]

        # x_cm [2, 128, (rh:14, ckpw:118)]: pixel col (rh, ck, pw) is the
        # CENTER value slab[c, PAD + ck*14 + rh, pw'] where the stage-1
        # conv is 1x1 at output pixel (ck*14+rh, pw-PAD): center padded
        # coords = (PAD + ck*14 + rh, pw) with pw = PAD+w... but we keep
        # all 62 cols for ck0 (cols pw=56..61 are junk pixels).
        # column ckpw maps to output pixel (14ck+rh, w=pw): center value
        # is slab[c, PAD + 14ck + rh, PAD + pw]; ck0 cols 56..61 are junk
        xcm = np.zeros((C, HQ, NCOL), BF16)
        xcm[:, :, 0:W] = slab[:, PAD:PAD + HQ, PAD:PAD + W]        # ck0
        xcm[:, :, PW:NCOL] = slab[:, PAD + HQ:PAD + HH, PAD:PAD + W]
        xcms.append(np.ascontiguousarray(xcm.reshape(C, NPIX)
                                         .reshape(2, 128, NPIX)))

        # xT [134, (r:20, cg:16, g:16)]: row 62ck+pw, 10 zero pad rows
        sg = slab.reshape(G, Cg, HH + 2 * PAD, PW)  # [g, cg, 34, 62]
        xt = np.zeros((134, PR, Cg, G), BF16)
        for ck in range(2):
            blk = sg[:, :, ck * HQ:ck * HQ + PR, :]  # [g, cg, 20, 62]
            xt[ck * PW:(ck + 1) * PW] = blk.transpose(3, 2, 1, 0)
        xts.append(np.ascontiguousarray(xt.reshape(134, PR * 256)))

    return xcms, xts, w1t, b1p, w2e


def kernel(inputs, W1, b1, gamma, beta, mean, var, W2, b2):
    global LAST_RESULT
    inputs = np.asarray(inputs, np.float32)
    if "nc" not in _CACHE:
        _CACHE["nc"] = _build_nc()
    nc = _CACHE["nc"]

    xcms, xts, w1t, b1p, w2e = _prep_host_inputs(
        inputs, np.asarray(W1, np.float32), np.asarray(b1, np.float32),
        np.asarray(gamma, np.float32), np.asarray(beta, np.float32),
        np.asarray(mean, np.float32), np.asarray(var, np.float32),
        np.asarray(W2, np.float32), np.asarray(b2, np.float32))

    in_maps = [{"xcm": xcms[core], "xt": xts[core], "w1t": w1t,
                "b1p": b1p, "w2e": w2e} for core in range(NCORES)]
    res = run_bass_kernel_spmd(nc, in_maps, list(range(NCORES)), trace=TRACE)
    LAST_RESULT = res

    out = np.empty((B, C, H, W), np.float32)
    for core in range(NCORES):
        b, hf = core // 2, core % 2
        o = res.results[core]["out"].astype(np.float32)  # [118, 3584] bf16
        o6 = o.reshape(NCOL, HQ, Cg, G)       # [ckpw, rh, cg, g]
        slab = np.empty((C, HH, W), np.float32)
        for ck in range(2):
            cols = o6[ck * PW:ck * PW + W]    # [56, 14, 16, 16]
            # -> [c = g*16+cg, rh, w]
            slab[:, ck * HQ:(ck + 1) * HQ, :] = (
                cols.transpose(3, 2, 1, 0).reshape(C, HQ, W))
        out[b, :, hf * HH:(hf + 1) * HH, :] = slab
    return out


# revision 3
# speedup vs baseline: 1.3614x; 1.1248x over previous
"""Involution (B=4, C=256, H=W=56, K=7, G=16, reduction=4) on 8 trn2 NeuronCores.

v2: pixel-major involution with zero-materialization weight broadcast.

Sharding: 8 shards = (batch b in 0..3) x (h-half hf in 0..1); each core
computes a [256, 28, 56] output slab.

Per-core layout (all bf16 in SBUF):
  - Pixel-major transposed input xT[124, (r:20, cg:16, g:16)]: partition
    p = 62*ck + pw where ck splits the 28 output rows into 2x14 and pw is
    the padded column (0..61); r is the padded row within the ck chunk
    (14+2*3=20); channel c = g*16+cg stored g-innermost so that per-group
    weights broadcast via a stride-0 middle AP dim.
  - Stage 1 (PE+ACT): t_ext[65, (rh:14, ckpw:118)] = [relu(W1p@x+b1p); 1]
    from channel-major x_cm; BN folded into W1p/b1p on host.
  - Stage 2 (PE+ACT): per rh: psum[118, 784] = t_ext[:,rh,:]^T @ W2e
    (cols n = gh*392 + k*8 + g8), one ACT copy -> wT[124,(rh,k,g)] bf16.
  - Stage 3 (DVE+GPSIMD): per tap k=(i,j), ONE op over the whole slab:
      acc += xT[j:j+118, i*256 : i*256+3584] * wT[0:118, :, k, :]-bcast
    The weight read uses AP [(rh,784),(cg,0),(g,1)] - stride-0 broadcast,
    verified to run in DVE 2x mode. ~11 taps run on gpsimd instead.
  - Merge accumulators -> f32, DMA out; host un-permutes.
"""

import numpy as np
import ml_dtypes
from contextlib import ExitStack

import concourse.bass as bass
import concourse.bacc as bacc
import concourse.tile as tile
from concourse import mybir
from concourse.bass_utils import run_bass_kernel_spmd

BF16 = ml_dtypes.bfloat16

B, C, H, W = 4, 256, 56, 56
KK, G, PAD = 7, 16, 3
Cr, Cg = 64, 16
EPS = 1e-5
HH = H // 2              # 28 rows per h-half shard
HQ = HH // 2             # 14 rows per sub-chunk (ck)
PW = W + 2 * PAD         # 62 padded width
PR = HQ + 2 * PAD        # 20 padded rows per sub-chunk
NCOL = PW + W            # 118 = ck0 cols 0..61 + ck1 cols 0..55
NPIX = HQ * NCOL         # 1652 stage-1/2 pixel columns
FREE3 = HQ * 256         # 3584 = stage-3 free size
NCORES = 8

# GPSIMD shares SBUF ports with the DVE: running Pool tensor_tensor ops
# concurrently with DVE taps collapses DVE throughput ~4.3x (measured), so
# stage 3 runs entirely on the DVE.
N_DVE_ACC = 4

_CACHE = {}
TRACE = False
LAST_RESULT = None


def _build_nc():
    nc = bacc.Bacc("TRN2", target_bir_lowering=False, debug=False,
                   num_devices=NCORES)

    f32 = mybir.dt.float32
    bf16 = mybir.dt.bfloat16

    xcm_d = nc.declare_dram_parameter("xcm", [2, 128, NPIX], bf16,
                                      isOutput=False)
    # 134 rows: 124 (ck,pw) rows + 10 zero rows so each shifted load
    # xTs[j] = xt[j : j+128] stays in range for j <= 6
    xt_d = nc.declare_dram_parameter("xt", [134, PR * 256], bf16,
                                     isOutput=False)
    # shift matrices S[j][kk, m] = 1 iff kk == m + j + 1 (for j-shifts 1..6)
    s_d = nc.declare_dram_parameter("smat", [128, 6, 118], bf16,
                                    isOutput=False)
    w1t_d = nc.declare_dram_parameter("w1t", [2, 128, Cr], bf16,
                                      isOutput=False)
    b1p_d = nc.declare_dram_parameter("b1p", [Cr, 1], f32, isOutput=False)
    w2e_d = nc.declare_dram_parameter("w2e", [Cr + 1, 784], bf16,
                                      isOutput=False)
    out_d = nc.declare_dram_parameter("out", [118, FREE3], bf16,
                                      isOutput=True)

    with tile.TileContext(nc) as tc, ExitStack() as ctx:
        const = ctx.enter_context(tc.tile_pool(name="const", bufs=1))
        xpool = ctx.enter_context(tc.tile_pool(name="x", bufs=1))
        tpool = ctx.enter_context(tc.tile_pool(name="t", bufs=1))
        wpool = ctx.enter_context(tc.tile_pool(name="w", bufs=1))

        # Only xTs[0] is loaded from DRAM (split across both HWDGE
        # queues); the 6 shifted copies are produced on-chip by the PE
        # (matmul with a shifted identity = partition shift), which keeps
        # the DMA engines quiet during the DVE tap window.
        x_cm = xpool.tile([128, 2, NPIX], bf16)
        nc.sync.dma_start(x_cm[:, 0, :], xcm_d[0])
        nc.scalar.dma_start(x_cm[:, 1, :], xcm_d[1])

        w1t_sb = const.tile([128, 2, Cr], bf16)
        for ch in range(2):
            nc.sync.dma_start(w1t_sb[:, ch, :], w1t_d[ch])
        b1p_sb = const.tile([Cr, 1], f32)
        nc.sync.dma_start(b1p_sb[:], b1p_d[:])
        w2e_sb = const.tile([Cr + 1, 784], bf16)
        nc.scalar.dma_start(w2e_sb[:], w2e_d[:])
        s_sb = const.tile([128, 6, 118], bf16)
        nc.gpsimd.dma_start(s_sb[:], s_d[:])

        xTs = []
        for j in range(KK):
            xtj = xpool.tile([128, PR * 256], bf16, name=f"xTs{j}")
            xTs.append(xtj)
        nc.sync.dma_start(xTs[0][0:64], xt_d[0:64])
        nc.scalar.dma_start(xTs[0][64:128], xt_d[64:128])

        # ---- stage 1: t_ext = [relu(W1p @ x + b1p); ones] ----
        t_ext = tpool.tile([Cr + 1, HQ, NCOL], bf16)
        nc.vector.memset(t_ext[Cr:Cr + 1, :, :], 1.0)
        t2 = t_ext[:].rearrange("p a b -> p (a b)")
        with tc.tile_pool(name="psum_t", bufs=2,
                          space=bass.MemorySpace.PSUM) as psum_t:
            NCH = 413  # 4 chunks of 413 = 1652 (<= 512 per psum bank)
            for q in range(4):
                pt = psum_t.tile([Cr, NCH], f32)
                for ch in range(2):
                    nc.tensor.matmul(pt[:], w1t_sb[:, ch, :],
                                     x_cm[:, ch, q * NCH:(q + 1) * NCH],
                                     start=(ch == 0), stop=(ch == 1))
                nc.scalar.activation(
                    t2[0:Cr, q * NCH:(q + 1) * NCH],
                    pt[:], mybir.ActivationFunctionType.Relu,
                    bias=b1p_sb[:], scale=1.0)

        # ---- stage 2: per-rh weight matmul -> wT[(rh, k, g)] ----
        # psum->sbuf copies alternate between ACT and the (still idle) DVE
        # so the copy wall halves.
        wT = wpool.tile([124, HQ, 49, G], bf16)
        with tc.tile_pool(name="psum_w", bufs=2,
                          space=bass.MemorySpace.PSUM) as psum_w:
            for rh in range(HQ):
                # [118, 2, 512]: each gh half starts on a psum bank boundary
                pw = psum_w.tile([NCOL, 2, 512], f32)
                for gh in range(2):
                    nc.tensor.matmul(pw[:, gh, 0:392],
                                     t_ext[:, rh, :],
                                     w2e_sb[:, gh * 392:(gh + 1) * 392],
                                     start=True, stop=True)
                # psum col (gh, k*8+g8) -> wT[p, rh, k, gh*8+g8]
                dst = wT[0:NCOL, rh, :, :].rearrange(
                    "p k (gh g8) -> p k gh g8", gh=2)
                src = pw[:, :, 0:392].rearrange(
                    "p gh (k g8) -> p k gh g8", k=49)
                if rh % 2 == 0:
                    nc.scalar.copy(dst, src)
                else:
                    nc.vector.tensor_copy(dst, src)

        # ---- on-chip j-shifts: xTs[j][m] = xTs[0][m+j] via PE matmul
        # with the shifted identity S[:, j-1, :]; runs on the otherwise
        # idle PE/ACT while the DVE crunches earlier-j taps.
        with tc.tile_pool(name="psum_s", bufs=2,
                          space=bass.MemorySpace.PSUM) as psum_s:
            for j in range(1, KK):
                for c0 in range(0, PR * 256, 1024):
                    ps = psum_s.tile([118, 2, 512], f32)
                    for h in range(2):
                        nc.tensor.matmul(
                            ps[:, h, :], s_sb[:, j - 1, :],
                            xTs[0][:, c0 + h * 512:c0 + (h + 1) * 512],
                            start=True, stop=True)
                    nc.scalar.copy(
                        xTs[j][0:118, c0:c0 + 1024],
                        ps[:].rearrange("p a b -> p (a b)"))

        # ---- stage 3: involution taps ----
        accp = ctx.enter_context(tc.tile_pool(name="acc", bufs=1))
        prodp = ctx.enter_context(tc.tile_pool(name="prod", bufs=6))

        daccs = [accp.tile([124, FREE3], bf16, name=f"dacc{m}")
                 for m in range(N_DVE_ACC)]

        nd = 0
        # j-major tap order: tap (i, j) consumes xTs[j]; the shift DMAs for
        # larger j complete while earlier-j taps run.
        for j in range(KK):
          for i in range(KK):
            k = i * KK + j
            src = xTs[j][0:118, i * 256:i * 256 + FREE3]
            wsl = wT[0:118, :, k, :].unsqueeze(2).broadcast_to(
                [118, HQ, Cg, G])
            src4 = src.rearrange("p (rh c) -> p rh c", rh=HQ).rearrange(
                "p rh (cg g) -> p rh cg g", g=G)
            m = nd % N_DVE_ACC
            ddst = daccs[m][0:118, :].rearrange(
                "p (rh cg g) -> p rh cg g", rh=HQ, g=G)
            if nd < N_DVE_ACC:
                nc.vector.tensor_mul(ddst, src4, wsl)
            else:
                dp = prodp.tile([124, FREE3], bf16, tag="dp")
                dp4 = dp[0:118, :].rearrange(
                    "p (rh cg g) -> p rh cg g", rh=HQ, g=G)
                nc.vector.tensor_mul(dp4, src4, wsl)
                nc.vector.tensor_add(daccs[m][0:118, :],
                                     daccs[m][0:118, :], dp[0:118, :])
            nd += 1

        # ---- merge (in-place, bf16; host converts to f32) + output ----
        # output DMA cost is per-partition-descriptor bound: split by
        # PARTITION range across the three queues so they run in parallel.
        nc.vector.tensor_add(daccs[0][0:118], daccs[0][0:118],
                             daccs[1][0:118])
        nc.vector.tensor_add(daccs[2][0:118], daccs[2][0:118],
                             daccs[3][0:118])
        nc.vector.tensor_add(daccs[0][0:118], daccs[0][0:118],
                             daccs[2][0:118])
        nc.sync.dma_start(out_d[0:40], daccs[0][0:40, :])
        nc.scalar.dma_start(out_d[40:79], daccs[0][40:79, :])
        nc.gpsimd.dma_start(out_d[79:118], daccs[0][79:118, :])

    nc.compile()
    return nc


def _prep_host_inputs(inputs, W1, b1, gamma, beta, mean, var, W2, b2):
    """Fold BN into W1/b1; build per-core transposed layouts."""
    scale = gamma / np.sqrt(var + EPS)
    shift = beta - mean * scale
    W1p = W1 * scale[:, None]
    b1p = (b1 * scale + shift).astype(np.float32).reshape(Cr, 1)
    w1t = np.ascontiguousarray(W1p.T.reshape(2, 128, Cr)).astype(BF16)

    # W2e [65, 784]: row 64 = b2; col n = gh*392 + k*8 + g8 for g = gh*8+g8
    w2full = np.concatenate([W2, b2[:, None]], axis=1)  # [784, 65]
    w2g = w2full.reshape(G, 49, Cr + 1)                 # [g, k, cr+1]
    w2e = w2g.reshape(2, 8, 49, Cr + 1).transpose(3, 0, 2, 1)  # [65,2,49,8]
    w2e = np.ascontiguousarray(w2e.reshape(Cr + 1, 784)).astype(BF16)

    # shift matrices: smat[kk, j-1, m] = 1 iff kk == m + j
    smat = np.zeros((128, 6, 118), BF16)
    for j in range(1, 7):
        for m in range(118):
            smat[m + j, j - 1, m] = 1.0

    xcms, xts = [], []
    for core in range(NCORES):
        b, hf = core // 2, core % 2
        slab = np.zeros((C, HH + 2 * PAD, PW), np.float32)
        r0 = hf * HH - PAD
        r1 = r0 + HH + 2 * PAD
        v0, v1 = max(r0, 0), min(r1, H)
        slab[:, v0 - r0:v1 - r0, PAD:PAD + W] = inputs[b, :, v0:v1, :]
        slab = slab.astype(BF16)  # [256, 34, 62]

        # x_cm [2, 128, (rh:14, ckpw:118)]: pixel col (rh, ck, pw) is the
        # CENTER value slab[c, PAD + ck*14 + rh, pw'] where the stage-1
        # conv is 1x1 at output pixel (ck*14+rh, pw-PAD): center padded
        # coords = (PAD + ck*14 + rh, pw) with pw = PAD+w... but we keep
        # all 62 cols for ck0 (cols pw=56..61 are junk pixels).
        # column ckpw maps to output pixel (14ck+rh, w=pw): center value
        # is slab[c, PAD + 14ck + rh, PAD + pw]; ck0 cols 56..61 are junk
        xcm = np.zeros((C, HQ, NCOL), BF16)
        xcm[:, :, 0:W] = slab[:, PAD:PAD + HQ, PAD:PAD + W]        # ck0
        xcm[:, :, PW:NCOL] = slab[:, PAD + HQ:PAD + HH, PAD:PAD + W]
        xcms.append(np.ascontiguousarray(xcm.reshape(C, NPIX)
                                         .reshape(2, 128, NPIX)))

        # xT [134, (r:20, cg:16, g:16)]: row 62ck+pw, 10 zero pad rows
        sg = slab.reshape(G, Cg, HH + 2 * PAD, PW)  # [g, cg, 34, 62]
        xt = np.zeros((134, PR, Cg, G), BF16)
        for ck in range(2):
            blk = sg[:, :, ck * HQ:ck * HQ + PR, :]  # [g, cg, 20, 62]
            xt[ck * PW:(ck + 1) * PW] = blk.transpose(3, 2, 1, 0)
        xts.append(np.ascontiguousarray(xt.reshape(134, PR * 256)))

    return xcms, xts, w1t, b1p, w2e, smat


def kernel(inputs, W1, b1, gamma, beta, mean, var, W2, b2):
    global LAST_RESULT
    inputs = np.asarray(inputs, np.float32)
    if "nc" not in _CACHE:
        _CACHE["nc"] = _build_nc()
    nc = _CACHE["nc"]

    xcms, xts, w1t, b1p, w2e, smat = _prep_host_inputs(
        inputs, np.asarray(W1, np.float32), np.asarray(b1, np.float32),
        np.asarray(gamma, np.float32), np.asarray(beta, np.float32),
        np.asarray(mean, np.float32), np.asarray(var, np.float32),
        np.asarray(W2, np.float32), np.asarray(b2, np.float32))

    in_maps = [{"xcm": xcms[core], "xt": xts[core], "w1t": w1t,
                "b1p": b1p, "w2e": w2e, "smat": smat}
               for core in range(NCORES)]
    res = run_bass_kernel_spmd(nc, in_maps, list(range(NCORES)), trace=TRACE)
    LAST_RESULT = res

    out = np.empty((B, C, H, W), np.float32)
    for core in range(NCORES):
        b, hf = core // 2, core % 2
        o = res.results[core]["out"].astype(np.float32)  # [118, 3584] bf16
        o6 = o.reshape(NCOL, HQ, Cg, G)       # [ckpw, rh, cg, g]
        slab = np.empty((C, HH, W), np.float32)
        for ck in range(2):
            cols = o6[ck * PW:ck * PW + W]    # [56, 14, 16, 16]
            # -> [c = g*16+cg, rh, w]
            slab[:, ck * HQ:(ck + 1) * HQ, :] = (
                cols.transpose(3, 2, 1, 0).reshape(C, HQ, W))
        out[b, :, hf * HH:(hf + 1) * HH, :] = slab
    return out


# revision 4
# speedup vs baseline: 1.3788x; 1.0128x over previous
"""Involution (B=4, C=256, H=W=56, K=7, G=16, reduction=4) on 8 trn2 NeuronCores.

v2: pixel-major involution with zero-materialization weight broadcast.

Sharding: 8 shards = (batch b in 0..3) x (h-half hf in 0..1); each core
computes a [256, 28, 56] output slab.

Per-core layout (all bf16 in SBUF):
  - Pixel-major transposed input xT[124, (r:20, cg:16, g:16)]: partition
    p = 62*ck + pw where ck splits the 28 output rows into 2x14 and pw is
    the padded column (0..61); r is the padded row within the ck chunk
    (14+2*3=20); channel c = g*16+cg stored g-innermost so that per-group
    weights broadcast via a stride-0 middle AP dim.
  - Stage 1 (PE+ACT): t_ext[65, (rh:14, ckpw:118)] = [relu(W1p@x+b1p); 1]
    from channel-major x_cm; BN folded into W1p/b1p on host.
  - Stage 2 (PE+ACT): per rh: psum[118, 784] = t_ext[:,rh,:]^T @ W2e
    (cols n = gh*392 + k*8 + g8), one ACT copy -> wT[124,(rh,k,g)] bf16.
  - Stage 3 (DVE+GPSIMD): per tap k=(i,j), ONE op over the whole slab:
      acc += xT[j:j+118, i*256 : i*256+3584] * wT[0:118, :, k, :]-bcast
    The weight read uses AP [(rh,784),(cg,0),(g,1)] - stride-0 broadcast,
    verified to run in DVE 2x mode. ~11 taps run on gpsimd instead.
  - Merge accumulators -> f32, DMA out; host un-permutes.
"""

import numpy as np
import ml_dtypes
from contextlib import ExitStack

import concourse.bass as bass
import concourse.bacc as bacc
import concourse.tile as tile
from concourse import mybir
from concourse.bass_utils import run_bass_kernel_spmd

BF16 = ml_dtypes.bfloat16

B, C, H, W = 4, 256, 56, 56
KK, G, PAD = 7, 16, 3
Cr, Cg = 64, 16
EPS = 1e-5
HH = H // 2              # 28 rows per h-half shard
HQ = HH // 2             # 14 rows per sub-chunk (ck)
PW = W + 2 * PAD         # 62 padded width
PR = HQ + 2 * PAD        # 20 padded rows per sub-chunk
NCOL = PW + W            # 118 = ck0 cols 0..61 + ck1 cols 0..55
NPIX = HQ * NCOL         # 1652 stage-1/2 pixel columns
FREE3 = HQ * 256         # 3584 = stage-3 free size
NCORES = 8

# GPSIMD shares SBUF ports with the DVE: running Pool tensor_tensor ops
# concurrently with DVE taps collapses DVE throughput ~4.3x (measured), so
# stage 3 runs entirely on the DVE.
N_DVE_ACC = 4

_CACHE = {}
TRACE = False
LAST_RESULT = None


def _build_nc():
    nc = bacc.Bacc("TRN2", target_bir_lowering=False, debug=False,
                   num_devices=NCORES)

    f32 = mybir.dt.float32
    bf16 = mybir.dt.bfloat16

    xcm_d = nc.declare_dram_parameter("xcm", [2, 128, NPIX], bf16,
                                      isOutput=False)
    # 134 rows: 124 (ck,pw) rows + 10 zero rows so each shifted load
    # xTs[j] = xt[j : j+128] stays in range for j <= 6
    xt_d = nc.declare_dram_parameter("xt", [134, PR * 256], bf16,
                                     isOutput=False)
    # shift matrices S[j][kk, m] = 1 iff kk == m + j + 1 (for j-shifts 1..6)
    s_d = nc.declare_dram_parameter("smat", [128, 6, 118], bf16,
                                    isOutput=False)
    w1t_d = nc.declare_dram_parameter("w1t", [2, 128, Cr], bf16,
                                      isOutput=False)
    b1p_d = nc.declare_dram_parameter("b1p", [Cr, 1], f32, isOutput=False)
    w2e_d = nc.declare_dram_parameter("w2e", [Cr + 1, 784], bf16,
                                      isOutput=False)
    out_d = nc.declare_dram_parameter("out", [118, FREE3], bf16,
                                      isOutput=True)

    with tile.TileContext(nc) as tc, ExitStack() as ctx:
        const = ctx.enter_context(tc.tile_pool(name="const", bufs=1))
        xpool = ctx.enter_context(tc.tile_pool(name="x", bufs=1))
        tpool = ctx.enter_context(tc.tile_pool(name="t", bufs=1))
        wpool = ctx.enter_context(tc.tile_pool(name="w", bufs=1))

        # Only xTs[0] is loaded from DRAM (split across both HWDGE
        # queues); the 6 shifted copies are produced on-chip by the PE
        # (matmul with a shifted identity = partition shift), which keeps
        # the DMA engines quiet during the DVE tap window.
        x_cm = xpool.tile([128, 2, NPIX], bf16)
        nc.sync.dma_start(x_cm[:, 0, :], xcm_d[0])
        nc.scalar.dma_start(x_cm[:, 1, :], xcm_d[1])

        w1t_sb = const.tile([128, 2, Cr], bf16)
        for ch in range(2):
            nc.sync.dma_start(w1t_sb[:, ch, :], w1t_d[ch])
        b1p_sb = const.tile([Cr, 1], f32)
        nc.sync.dma_start(b1p_sb[:], b1p_d[:])
        w2e_sb = const.tile([Cr + 1, 784], bf16)
        nc.scalar.dma_start(w2e_sb[:], w2e_d[:])
        s_sb = const.tile([128, 6, 118], bf16)
        nc.gpsimd.dma_start(s_sb[:], s_d[:])

        xTs = []
        for j in range(KK):
            xtj = xpool.tile([128, PR * 256], bf16, name=f"xTs{j}")
            xTs.append(xtj)
        nc.sync.dma_start(xTs[0][0:64], xt_d[0:64])
        nc.scalar.dma_start(xTs[0][64:128], xt_d[64:128])

        # ---- stage 1: t_ext = [relu(W1p @ x + b1p); ones] ----
        t_ext = tpool.tile([Cr + 1, HQ, NCOL], bf16)
        nc.vector.memset(t_ext[Cr:Cr + 1, :, :], 1.0)
        t2 = t_ext[:].rearrange("p a b -> p (a b)")
        with tc.tile_pool(name="psum_t", bufs=2,
                          space=bass.MemorySpace.PSUM) as psum_t:
            NCH = 413  # 4 chunks of 413 = 1652 (<= 512 per psum bank)
            for q in range(4):
                pt = psum_t.tile([Cr, NCH], f32)
                for ch in range(2):
                    nc.tensor.matmul(pt[:], w1t_sb[:, ch, :],
                                     x_cm[:, ch, q * NCH:(q + 1) * NCH],
                                     start=(ch == 0), stop=(ch == 1))
                nc.scalar.activation(
                    t2[0:Cr, q * NCH:(q + 1) * NCH],
                    pt[:], mybir.ActivationFunctionType.Relu,
                    bias=b1p_sb[:], scale=1.0)

        # ---- stage 2: per-rh weight matmul -> wT[(rh, k, g)] ----
        # psum->sbuf copies alternate between ACT and the (still idle) DVE
        # so the copy wall halves.
        wT = wpool.tile([124, HQ, 49, G], bf16)
        with tc.tile_pool(name="psum_w", bufs=3,
                          space=bass.MemorySpace.PSUM) as psum_w:
            for rh in range(HQ):
                # [118, 2, 512]: each gh half starts on a psum bank boundary
                pw = psum_w.tile([NCOL, 2, 512], f32)
                for gh in range(2):
                    nc.tensor.matmul(pw[:, gh, 0:392],
                                     t_ext[:, rh, :],
                                     w2e_sb[:, gh * 392:(gh + 1) * 392],
                                     start=True, stop=True)
                # psum col (gh, k*8+g8) -> wT[p, rh, k, gh*8+g8]
                dst = wT[0:NCOL, rh, :, :].rearrange(
                    "p k (gh g8) -> p k gh g8", gh=2)
                src = pw[:, :, 0:392].rearrange(
                    "p gh (k g8) -> p k gh g8", k=49)
                if rh % 2 == 0:
                    nc.scalar.copy(dst, src)
                else:
                    nc.vector.tensor_copy(dst, src)

        # ---- on-chip j-shifts: xTs[j][m] = xTs[0][m+j] via PE matmul
        # with the shifted identity S[:, j-1, :]; runs on the otherwise
        # idle PE/ACT while the DVE crunches earlier-j taps.
        with tc.tile_pool(name="psum_s", bufs=1,
                          space=bass.MemorySpace.PSUM) as psum_s:
            for j in range(1, KK):
                for c0 in range(0, PR * 256, 1024):
                    ps = psum_s.tile([118, 2, 512], f32)
                    for h in range(2):
                        nc.tensor.matmul(
                            ps[:, h, :], s_sb[:, j - 1, :],
                            xTs[0][:, c0 + h * 512:c0 + (h + 1) * 512],
                            start=True, stop=True)
                    nc.scalar.copy(
                        xTs[j][0:118, c0:c0 + 1024],
                        ps[:].rearrange("p a b -> p (a b)"))

        # ---- stage 3: involution taps ----
        accp = ctx.enter_context(tc.tile_pool(name="acc", bufs=1))
        prodp = ctx.enter_context(tc.tile_pool(name="prod", bufs=6))

        daccs = [accp.tile([124, FREE3], bf16, name=f"dacc{m}")
                 for m in range(N_DVE_ACC)]

        nd = 0
        # j-major tap order: tap (i, j) consumes xTs[j]; the shift DMAs for
        # larger j complete while earlier-j taps run.
        for j in range(KK):
          for i in range(KK):
            k = i * KK + j
            if nd == 48:
                nc.vector.tensor_add(daccs[2][0:118], daccs[2][0:118],
                                     daccs[3][0:118])
            src = xTs[j][0:118, i * 256:i * 256 + FREE3]
            wsl = wT[0:118, :, k, :].unsqueeze(2).broadcast_to(
                [118, HQ, Cg, G])
            src4 = src.rearrange("p (rh c) -> p rh c", rh=HQ).rearrange(
                "p rh (cg g) -> p rh cg g", g=G)
            m = nd % N_DVE_ACC
            ddst = daccs[m][0:118, :].rearrange(
                "p (rh cg g) -> p rh cg g", rh=HQ, g=G)
            if nd < N_DVE_ACC:
                nc.vector.tensor_mul(ddst, src4, wsl)
            else:
                dp = prodp.tile([124, FREE3], bf16, tag="dp")
                dp4 = dp[0:118, :].rearrange(
                    "p (rh cg g) -> p rh cg g", rh=HQ, g=G)
                nc.vector.tensor_mul(dp4, src4, wsl)
                nc.vector.tensor_add(daccs[m][0:118, :],
                                     daccs[m][0:118, :], dp[0:118, :])
            nd += 1

        # ---- merge (in-place, bf16; host converts to f32) + output ----
        # output DMA cost is per-partition-descriptor bound: split by
        # PARTITION range across the three queues so they run in parallel.
        nc.vector.tensor_add(daccs[0][0:118], daccs[0][0:118],
                             daccs[1][0:118])
        nc.vector.tensor_add(daccs[0][0:118], daccs[0][0:118],
                             daccs[2][0:118])
        nc.sync.dma_start(out_d[0:40], daccs[0][0:40, :])
        nc.scalar.dma_start(out_d[40:79], daccs[0][40:79, :])
        nc.gpsimd.dma_start(out_d[79:118], daccs[0][79:118, :])

    nc.compile()
    return nc


def _prep_host_inputs(inputs, W1, b1, gamma, beta, mean, var, W2, b2):
    """Fold BN into W1/b1; build per-core transposed layouts."""
    scale = gamma / np.sqrt(var + EPS)
    shift = beta - mean * scale
    W1p = W1 * scale[:, None]
    b1p = (b1 * scale + shift).astype(np.float32).reshape(Cr, 1)
    w1t = np.ascontiguousarray(W1p.T.reshape(2, 128, Cr)).astype(BF16)

    # W2e [65, 784]: row 64 = b2; col n = gh*392 + k*8 + g8 for g = gh*8+g8
    w2full = np.concatenate([W2, b2[:, None]], axis=1)  # [784, 65]
    w2g = w2full.reshape(G, 49, Cr + 1)                 # [g, k, cr+1]
    w2e = w2g.reshape(2, 8, 49, Cr + 1).transpose(3, 0, 2, 1)  # [65,2,49,8]
    w2e = np.ascontiguousarray(w2e.reshape(Cr + 1, 784)).astype(BF16)

    # shift matrices: smat[kk, j-1, m] = 1 iff kk == m + j
    smat = np.zeros((128, 6, 118), BF16)
    for j in range(1, 7):
        for m in range(118):
            smat[m + j, j - 1, m] = 1.0

    xcms, xts = [], []
    for core in range(NCORES):
        b, hf = core // 2, core % 2
        slab = np.zeros((C, HH + 2 * PAD, PW), np.float32)
        r0 = hf * HH - PAD
        r1 = r0 + HH + 2 * PAD
        v0, v1 = max(r0, 0), min(r1, H)
        slab[:, v0 - r0:v1 - r0, PAD:PAD + W] = inputs[b, :, v0:v1, :]
        slab = slab.astype(BF16)  # [256, 34, 62]

        # x_cm [2, 128, (rh:14, ckpw:118)]: pixel col (rh, ck, pw) is the
        # CENTER value slab[c, PAD + ck*14 + rh, pw'] where the stage-1
        # conv is 1x1 at output pixel (ck*14+rh, pw-PAD): center padded
        # coords = (PAD + ck*14 + rh, pw) with pw = PAD+w... but we keep
        # all 62 cols for ck0 (cols pw=56..61 are junk pixels).
        # column ckpw maps to output pixel (14ck+rh, w=pw): center value
        # is slab[c, PAD + 14ck + rh, PAD + pw]; ck0 cols 56..61 are junk
        xcm = np.zeros((C, HQ, NCOL), BF16)
        xcm[:, :, 0:W] = slab[:, PAD:PAD + HQ, PAD:PAD + W]        # ck0
        xcm[:, :, PW:NCOL] = slab[:, PAD + HQ:PAD + HH, PAD:PAD + W]
        xcms.append(np.ascontiguousarray(xcm.reshape(C, NPIX)
                                         .reshape(2, 128, NPIX)))

        # xT [134, (r:20, cg:16, g:16)]: row 62ck+pw, 10 zero pad rows
        sg = slab.reshape(G, Cg, HH + 2 * PAD, PW)  # [g, cg, 34, 62]
        xt = np.zeros((134, PR, Cg, G), BF16)
        for ck in range(2):
            blk = sg[:, :, ck * HQ:ck * HQ + PR, :]  # [g, cg, 20, 62]
            xt[ck * PW:(ck + 1) * PW] = blk.transpose(3, 2, 1, 0)
        xts.append(np.ascontiguousarray(xt.reshape(134, PR * 256)))

    return xcms, xts, w1t, b1p, w2e, smat


def kernel(inputs, W1, b1, gamma, beta, mean, var, W2, b2):
    global LAST_RESULT
    inputs = np.asarray(inputs, np.float32)
    if "nc" not in _CACHE:
        _CACHE["nc"] = _build_nc()
    nc = _CACHE["nc"]

    xcms, xts, w1t, b1p, w2e, smat = _prep_host_inputs(
        inputs, np.asarray(W1, np.float32), np.asarray(b1, np.float32),
        np.asarray(gamma, np.float32), np.asarray(beta, np.float32),
        np.asarray(mean, np.float32), np.asarray(var, np.float32),
        np.asarray(W2, np.float32), np.asarray(b2, np.float32))

    in_maps = [{"xcm": xcms[core], "xt": xts[core], "w1t": w1t,
                "b1p": b1p, "w2e": w2e, "smat": smat}
               for core in range(NCORES)]
    res = run_bass_kernel_spmd(nc, in_maps, list(range(NCORES)), trace=TRACE)
    LAST_RESULT = res

    out = np.empty((B, C, H, W), np.float32)
    for core in range(NCORES):
        b, hf = core // 2, core % 2
        o = res.results[core]["out"].astype(np.float32)  # [118, 3584] bf16
        o6 = o.reshape(NCOL, HQ, Cg, G)       # [ckpw, rh, cg, g]
        slab = np.empty((C, HH, W), np.float32)
        for ck in range(2):
            cols = o6[ck * PW:ck * PW + W]    # [56, 14, 16, 16]
            # -> [c = g*16+cg, rh, w]
            slab[:, ck * HQ:(ck + 1) * HQ, :] = (
                cols.transpose(3, 2, 1, 0).reshape(C, HQ, W))
        out[b, :, hf * HH:(hf + 1) * HH, :] = slab
    return out


# revision 5
# speedup vs baseline: 1.5688x; 1.1378x over previous
"""Involution (B=4, C=256, H=W=56, K=7, G=16, reduction=4) on 8 trn2 NeuronCores.

v2: pixel-major involution with zero-materialization weight broadcast.

Sharding: 8 shards = (batch b in 0..3) x (h-half hf in 0..1); each core
computes a [256, 28, 56] output slab.

Per-core layout (all bf16 in SBUF):
  - Pixel-major transposed input xT[124, (r:20, cg:16, g:16)]: partition
    p = 62*ck + pw where ck splits the 28 output rows into 2x14 and pw is
    the padded column (0..61); r is the padded row within the ck chunk
    (14+2*3=20); channel c = g*16+cg stored g-innermost so that per-group
    weights broadcast via a stride-0 middle AP dim.
  - Stage 1 (PE+ACT): t_ext[65, (rh:14, ckpw:118)] = [relu(W1p@x+b1p); 1]
    from channel-major x_cm; BN folded into W1p/b1p on host.
  - Stage 2 (PE+ACT): per rh: psum[118, 784] = t_ext[:,rh,:]^T @ W2e
    (cols n = gh*392 + k*8 + g8), one ACT copy -> wT[124,(rh,k,g)] bf16.
  - Stage 3 (DVE+GPSIMD): per tap k=(i,j), ONE op over the whole slab:
      acc += xT[j:j+118, i*256 : i*256+3584] * wT[0:118, :, k, :]-bcast
    The weight read uses AP [(rh,784),(cg,0),(g,1)] - stride-0 broadcast,
    verified to run in DVE 2x mode. ~11 taps run on gpsimd instead.
  - Merge accumulators -> f32, DMA out; host un-permutes.
"""

import numpy as np
import ml_dtypes
from contextlib import ExitStack

import concourse.bass as bass
import concourse.bacc as bacc
import concourse.tile as tile
from concourse import mybir
from concourse.bass_utils import run_bass_kernel_spmd

BF16 = ml_dtypes.bfloat16

B, C, H, W = 4, 256, 56, 56
KK, G, PAD = 7, 16, 3
Cr, Cg = 64, 16
EPS = 1e-5
HH = H // 2              # 28 rows per h-half shard
HQ = HH // 2             # 14 rows per sub-chunk (ck)
PW = W + 2 * PAD         # 62 padded width
PR = HQ + 2 * PAD        # 20 padded rows per sub-chunk
NCOL = PW + W            # 118 = ck0 cols 0..61 + ck1 cols 0..55
NPIX = HQ * NCOL         # 1652 stage-1/2 pixel columns
FREE3 = HQ * 256         # 3584 = stage-3 free size
NCORES = 8

# GPSIMD shares SBUF ports with the DVE: running Pool tensor_tensor ops
# concurrently with DVE taps collapses DVE throughput ~4.3x (measured), so
# stage 3 runs entirely on the DVE.
N_DVE_ACC = 4

_CACHE = {}
TRACE = False
LAST_RESULT = None


def _build_nc():
    nc = bacc.Bacc("TRN2", target_bir_lowering=False, debug=False,
                   num_devices=NCORES)

    f32 = mybir.dt.float32
    bf16 = mybir.dt.bfloat16

    xcm_d = nc.declare_dram_parameter("xcm", [2, 128, NPIX], bf16,
                                      isOutput=False)
    # 134 rows: 124 (ck,pw) rows + 10 zero rows so each shifted load
    # xTs[j] = xt[j : j+128] stays in range for j <= 6
    xt_d = nc.declare_dram_parameter("xt", [134, PR * 256], bf16,
                                     isOutput=False)
    w1t_d = nc.declare_dram_parameter("w1t", [2, 128, Cr], bf16,
                                      isOutput=False)
    b1p_d = nc.declare_dram_parameter("b1p", [Cr, 1], f32, isOutput=False)
    w2e_d = nc.declare_dram_parameter("w2e", [Cr + 1, 784], bf16,
                                      isOutput=False)
    out_d = nc.declare_dram_parameter("out", [118, FREE3], bf16,
                                      isOutput=True)

    with tile.TileContext(nc) as tc, ExitStack() as ctx:
        const = ctx.enter_context(tc.tile_pool(name="const", bufs=1))
        xpool = ctx.enter_context(tc.tile_pool(name="x", bufs=1))
        tpool = ctx.enter_context(tc.tile_pool(name="t", bufs=1))
        wpool = ctx.enter_context(tc.tile_pool(name="w", bufs=1))

        # Only xTs[0] is loaded from DRAM (split across both HWDGE
        # queues); the 6 shifted copies are produced on-chip by the PE
        # (matmul with a shifted identity = partition shift), which keeps
        # the DMA engines quiet during the DVE tap window.
        x_cm = xpool.tile([128, 2, NPIX], bf16)
        nc.sync.dma_start(x_cm[:, 0, :], xcm_d[0])
        nc.scalar.dma_start(x_cm[:, 1, :], xcm_d[1])

        w1t_sb = const.tile([128, 2, Cr], bf16)
        for ch in range(2):
            nc.sync.dma_start(w1t_sb[:, ch, :], w1t_d[ch])
        b1p_sb = const.tile([Cr, 1], f32)
        nc.sync.dma_start(b1p_sb[:], b1p_d[:])
        w2e_sb = const.tile([Cr + 1, 784], bf16)
        nc.scalar.dma_start(w2e_sb[:], w2e_d[:])
        # shift matrices S[kk, j-1, m] = 1 iff kk == m + j, built on-chip
        # (keeps the single shared DMA engine free for the x loads)
        s_sb = const.tile([128, 6, 118], bf16)
        nc.gpsimd.memset(s_sb[:], 1.0)
        nc.gpsimd.affine_select(
            s_sb[:], s_sb[:], pattern=[[-1, 6], [-1, 118]],
            compare_op=mybir.AluOpType.is_equal, fill=0.0, base=-1,
            channel_multiplier=1)

        xTs = []
        for j in range(KK):
            xtj = xpool.tile([128, PR * 256], bf16, name=f"xTs{j}")
            xTs.append(xtj)
        nc.sync.dma_start(xTs[0][0:64], xt_d[0:64])
        nc.scalar.dma_start(xTs[0][64:128], xt_d[64:128])

        # ---- stage 1: t_ext = [relu(W1p @ x + b1p); ones] ----
        t_ext = tpool.tile([Cr + 1, HQ, NCOL], bf16)
        nc.vector.memset(t_ext[Cr:Cr + 1, :, :], 1.0)
        t2 = t_ext[:].rearrange("p a b -> p (a b)")
        with tc.tile_pool(name="psum_t", bufs=2,
                          space=bass.MemorySpace.PSUM) as psum_t:
            NCH = 413  # 4 chunks of 413 = 1652 (<= 512 per psum bank)
            for q in range(4):
                pt = psum_t.tile([Cr, NCH], f32)
                for ch in range(2):
                    nc.tensor.matmul(pt[:], w1t_sb[:, ch, :],
                                     x_cm[:, ch, q * NCH:(q + 1) * NCH],
                                     start=(ch == 0), stop=(ch == 1))
                nc.scalar.activation(
                    t2[0:Cr, q * NCH:(q + 1) * NCH],
                    pt[:], mybir.ActivationFunctionType.Relu,
                    bias=b1p_sb[:], scale=1.0)

        # ---- stage 2: per-rh weight matmul -> wT[(rh, k, g)] ----
        # psum->sbuf copies alternate between ACT and the (still idle) DVE
        # so the copy wall halves.
        wT = wpool.tile([124, HQ, 49, G], bf16)
        with tc.tile_pool(name="psum_w", bufs=3,
                          space=bass.MemorySpace.PSUM) as psum_w:
            for rh in range(HQ):
                # [118, 2, 512]: each gh half starts on a psum bank boundary
                pw = psum_w.tile([NCOL, 2, 512], f32)
                for gh in range(2):
                    nc.tensor.matmul(pw[:, gh, 0:392],
                                     t_ext[:, rh, :],
                                     w2e_sb[:, gh * 392:(gh + 1) * 392],
                                     start=True, stop=True)
                # psum col (gh, k*8+g8) -> wT[p, rh, k, gh*8+g8]
                dst = wT[0:NCOL, rh, :, :].rearrange(
                    "p k (gh g8) -> p k gh g8", gh=2)
                src = pw[:, :, 0:392].rearrange(
                    "p gh (k g8) -> p k gh g8", k=49)
                if rh % 2 == 0:
                    nc.scalar.copy(dst, src)
                else:
                    nc.vector.tensor_copy(dst, src)

        # ---- on-chip j-shifts: xTs[j][m] = xTs[0][m+j] via PE matmul
        # with the shifted identity S[:, j-1, :]; runs on the otherwise
        # idle PE/ACT while the DVE crunches earlier-j taps.
        with tc.tile_pool(name="psum_s", bufs=1,
                          space=bass.MemorySpace.PSUM) as psum_s:
            for j in range(1, KK):
                for c0 in range(0, PR * 256, 1024):
                    ps = psum_s.tile([118, 2, 512], f32)
                    for h in range(2):
                        nc.tensor.matmul(
                            ps[:, h, :], s_sb[:, j - 1, :],
                            xTs[0][:, c0 + h * 512:c0 + (h + 1) * 512],
                            start=True, stop=True)
                    nc.scalar.copy(
                        xTs[j][0:118, c0:c0 + 1024],
                        ps[:].rearrange("p a b -> p (a b)"))

        # ---- stage 3: involution taps ----
        accp = ctx.enter_context(tc.tile_pool(name="acc", bufs=1))
        prodp = ctx.enter_context(tc.tile_pool(name="prod", bufs=6))

        daccs = [accp.tile([124, FREE3], bf16, name=f"dacc{m}")
                 for m in range(N_DVE_ACC)]

        nd = 0
        # j-major tap order: tap (i, j) consumes xTs[j]; the shift DMAs for
        # larger j complete while earlier-j taps run.
        for j in range(KK):
          for i in range(KK):
            k = i * KK + j
            if nd == 48:
                nc.vector.tensor_add(daccs[2][0:118], daccs[2][0:118],
                                     daccs[3][0:118])
            src = xTs[j][0:118, i * 256:i * 256 + FREE3]
            wsl = wT[0:118, :, k, :].unsqueeze(2).broadcast_to(
                [118, HQ, Cg, G])
            src4 = src.rearrange("p (rh c) -> p rh c", rh=HQ).rearrange(
                "p rh (cg g) -> p rh cg g", g=G)
            m = nd % N_DVE_ACC
            ddst = daccs[m][0:118, :].rearrange(
                "p (rh cg g) -> p rh cg g", rh=HQ, g=G)
            if nd < N_DVE_ACC:
                nc.vector.tensor_mul(ddst, src4, wsl)
            else:
                dp = prodp.tile([124, FREE3], bf16, tag="dp")
                dp4 = dp[0:118, :].rearrange(
                    "p (rh cg g) -> p rh cg g", rh=HQ, g=G)
                nc.vector.tensor_mul(dp4, src4, wsl)
                nc.vector.tensor_add(daccs[m][0:118, :],
                                     daccs[m][0:118, :], dp[0:118, :])
            nd += 1

        # ---- merge (in-place, bf16; host converts to f32) + output ----
        # output DMA cost is per-partition-descriptor bound: split by
        # PARTITION range across the three queues so they run in parallel.
        nc.vector.tensor_add(daccs[0][0:118], daccs[0][0:118],
                             daccs[1][0:118])
        nc.vector.tensor_add(daccs[0][0:118], daccs[0][0:118],
                             daccs[2][0:118])
        nc.sync.dma_start(out_d[0:40], daccs[0][0:40, :])
        nc.scalar.dma_start(out_d[40:79], daccs[0][40:79, :])
        nc.gpsimd.dma_start(out_d[79:118], daccs[0][79:118, :])

    nc.compile()
    return nc


def _prep_host_inputs(inputs, W1, b1, gamma, beta, mean, var, W2, b2):
    """Fold BN into W1/b1; build per-core transposed layouts."""
    scale = gamma / np.sqrt(var + EPS)
    shift = beta - mean * scale
    W1p = W1 * scale[:, None]
    b1p = (b1 * scale + shift).astype(np.float32).reshape(Cr, 1)
    w1t = np.ascontiguousarray(W1p.T.reshape(2, 128, Cr)).astype(BF16)

    # W2e [65, 784]: row 64 = b2; col n = gh*392 + k*8 + g8 for g = gh*8+g8
    w2full = np.concatenate([W2, b2[:, None]], axis=1)  # [784, 65]
    w2g = w2full.reshape(G, 49, Cr + 1)                 # [g, k, cr+1]
    w2e = w2g.reshape(2, 8, 49, Cr + 1).transpose(3, 0, 2, 1)  # [65,2,49,8]
    w2e = np.ascontiguousarray(w2e.reshape(Cr + 1, 784)).astype(BF16)

    xcms, xts = [], []
    for core in range(NCORES):
        b, hf = core // 2, core % 2
        slab = np.zeros((C, HH + 2 * PAD, PW), np.float32)
        r0 = hf * HH - PAD
        r1 = r0 + HH + 2 * PAD
        v0, v1 = max(r0, 0), min(r1, H)
        slab[:, v0 - r0:v1 - r0, PAD:PAD + W] = inputs[b, :, v0:v1, :]
        slab = slab.astype(BF16)  # [256, 34, 62]

        # x_cm [2, 128, (rh:14, ckpw:118)]: pixel col (rh, ck, pw) is the
        # CENTER value slab[c, PAD + ck*14 + rh, pw'] where the stage-1
        # conv is 1x1 at output pixel (ck*14+rh, pw-PAD): center padded
        # coords = (PAD + ck*14 + rh, pw) with pw = PAD+w... but we keep
        # all 62 cols for ck0 (cols pw=56..61 are junk pixels).
        # column ckpw maps to output pixel (14ck+rh, w=pw): center value
        # is slab[c, PAD + 14ck + rh, PAD + pw]; ck0 cols 56..61 are junk
        xcm = np.zeros((C, HQ, NCOL), BF16)
        xcm[:, :, 0:W] = slab[:, PAD:PAD + HQ, PAD:PAD + W]        # ck0
        xcm[:, :, PW:NCOL] = slab[:, PAD + HQ:PAD + HH, PAD:PAD + W]
        xcms.append(np.ascontiguousarray(xcm.reshape(C, NPIX)
                                         .reshape(2, 128, NPIX)))

        # xT [134, (r:20, cg:16, g:16)]: row 62ck+pw, 10 zero pad rows
        sg = slab.reshape(G, Cg, HH + 2 * PAD, PW)  # [g, cg, 34, 62]
        xt = np.zeros((134, PR, Cg, G), BF16)
        for ck in range(2):
            blk = sg[:, :, ck * HQ:ck * HQ + PR, :]  # [g, cg, 20, 62]
            xt[ck * PW:(ck + 1) * PW] = blk.transpose(3, 2, 1, 0)
        xts.append(np.ascontiguousarray(xt.reshape(134, PR * 256)))

    return xcms, xts, w1t, b1p, w2e


def kernel(inputs, W1, b1, gamma, beta, mean, var, W2, b2):
    global LAST_RESULT
    inputs = np.asarray(inputs, np.float32)
    if "nc" not in _CACHE:
        _CACHE["nc"] = _build_nc()
    nc = _CACHE["nc"]

    xcms, xts, w1t, b1p, w2e = _prep_host_inputs(
        inputs, np.asarray(W1, np.float32), np.asarray(b1, np.float32),
        np.asarray(gamma, np.float32), np.asarray(beta, np.float32),
        np.asarray(mean, np.float32), np.asarray(var, np.float32),
        np.asarray(W2, np.float32), np.asarray(b2, np.float32))

    in_maps = [{"xcm": xcms[core], "xt": xts[core], "w1t": w1t,
                "b1p": b1p, "w2e": w2e} for core in range(NCORES)]
    res = run_bass_kernel_spmd(nc, in_maps, list(range(NCORES)), trace=TRACE)
    LAST_RESULT = res

    out = np.empty((B, C, H, W), np.float32)
    for core in range(NCORES):
        b, hf = core // 2, core % 2
        o = res.results[core]["out"].astype(np.float32)  # [118, 3584] bf16
        o6 = o.reshape(NCOL, HQ, Cg, G)       # [ckpw, rh, cg, g]
        slab = np.empty((C, HH, W), np.float32)
        for ck in range(2):
            cols = o6[ck * PW:ck * PW + W]    # [56, 14, 16, 16]
            # -> [c = g*16+cg, rh, w]
            slab[:, ck * HQ:(ck + 1) * HQ, :] = (
                cols.transpose(3, 2, 1, 0).reshape(C, HQ, W))
        out[b, :, hf * HH:(hf + 1) * HH, :] = slab
    return out


# revision 6
# speedup vs baseline: 1.9614x; 1.2503x over previous
"""Involution (B=4, C=256, H=W=56, K=7, G=16, reduction=4) on 8 trn2 NeuronCores.

v2: pixel-major involution with zero-materialization weight broadcast.

Sharding: 8 shards = (batch b in 0..3) x (h-half hf in 0..1); each core
computes a [256, 28, 56] output slab.

Per-core layout (all bf16 in SBUF):
  - Pixel-major transposed input xT[124, (r:20, cg:16, g:16)]: partition
    p = 62*ck + pw where ck splits the 28 output rows into 2x14 and pw is
    the padded column (0..61); r is the padded row within the ck chunk
    (14+2*3=20); channel c = g*16+cg stored g-innermost so that per-group
    weights broadcast via a stride-0 middle AP dim.
  - Stage 1 (PE+ACT): t_ext[65, (rh:14, ckpw:118)] = [relu(W1p@x+b1p); 1]
    from channel-major x_cm; BN folded into W1p/b1p on host.
  - Stage 2 (PE+ACT): per rh: psum[118, 784] = t_ext[:,rh,:]^T @ W2e
    (cols n = gh*392 + k*8 + g8), one ACT copy -> wT[124,(rh,k,g)] bf16.
  - Stage 3 (DVE+GPSIMD): per tap k=(i,j), ONE op over the whole slab:
      acc += xT[j:j+118, i*256 : i*256+3584] * wT[0:118, :, k, :]-bcast
    The weight read uses AP [(rh,784),(cg,0),(g,1)] - stride-0 broadcast,
    verified to run in DVE 2x mode. ~11 taps run on gpsimd instead.
  - Merge accumulators -> f32, DMA out; host un-permutes.
"""

import numpy as np
import ml_dtypes
from contextlib import ExitStack

import concourse.bass as bass
import concourse.bacc as bacc
import concourse.tile as tile
from concourse import mybir
from concourse.bass_utils import run_bass_kernel_spmd

BF16 = ml_dtypes.bfloat16

B, C, H, W = 4, 256, 56, 56
KK, G, PAD = 7, 16, 3
Cr, Cg = 64, 16
EPS = 1e-5
HH = H // 2              # 28 rows per h-half shard
HQ = HH // 2             # 14 rows per sub-chunk (ck)
PW = W + 2 * PAD         # 62 padded width
PR = HQ + 2 * PAD        # 20 padded rows per sub-chunk
NCOL = PW + W            # 118 = ck0 cols 0..61 + ck1 cols 0..55
NPIX = HQ * NCOL         # 1652 stage-1/2 pixel columns
FREE3 = HQ * 256         # 3584 = stage-3 free size
NCORES = 8

# GPSIMD shares SBUF ports with the DVE: running Pool tensor_tensor ops
# concurrently with DVE taps collapses DVE throughput ~4.3x (measured), so
# stage 3 runs entirely on the DVE.
N_DVE_ACC = 4

_CACHE = {}
TRACE = False
LAST_RESULT = None


def _build_nc():
    nc = bacc.Bacc("TRN2", target_bir_lowering=False, debug=False,
                   num_devices=NCORES)

    f32 = mybir.dt.float32
    bf16 = mybir.dt.bfloat16

    xcm_d = nc.declare_dram_parameter("xcm", [2, 128, NPIX], bf16,
                                      isOutput=False)
    # 134 rows: 124 (ck,pw) rows + 10 zero rows so each shifted load
    # xTs[j] = xt[j : j+128] stays in range for j <= 6
    xt_d = nc.declare_dram_parameter("xt", [134, PR * 256], bf16,
                                     isOutput=False)
    w1t_d = nc.declare_dram_parameter("w1t", [2, 128, Cr], bf16,
                                      isOutput=False)
    b1p_d = nc.declare_dram_parameter("b1p", [Cr, 1], f32, isOutput=False)
    w2e_d = nc.declare_dram_parameter("w2e", [Cr + 1, 784], bf16,
                                      isOutput=False)
    out_d = nc.declare_dram_parameter("out", [118, FREE3], bf16,
                                      isOutput=True)

    with tile.TileContext(nc) as tc, ExitStack() as ctx:
        const = ctx.enter_context(tc.tile_pool(name="const", bufs=1))
        xpool = ctx.enter_context(tc.tile_pool(name="x", bufs=1))
        tpool = ctx.enter_context(tc.tile_pool(name="t", bufs=1))
        wpool = ctx.enter_context(tc.tile_pool(name="w", bufs=1))

        # Only xTs[0] is loaded from DRAM (split across both HWDGE
        # queues); the 6 shifted copies are produced on-chip by the PE
        # (matmul with a shifted identity = partition shift), which keeps
        # the DMA engines quiet during the DVE tap window.
        x_cm = xpool.tile([128, 2, NPIX], bf16)
        nc.sync.dma_start(x_cm[:, 0, :], xcm_d[0])
        nc.scalar.dma_start(x_cm[:, 1, :], xcm_d[1])

        w1t_sb = const.tile([128, 2, Cr], bf16)
        for ch in range(2):
            nc.sync.dma_start(w1t_sb[:, ch, :], w1t_d[ch])
        b1p_sb = const.tile([Cr, 1], f32)
        nc.sync.dma_start(b1p_sb[:], b1p_d[:])
        w2e_sb = const.tile([Cr + 1, 784], bf16)
        nc.scalar.dma_start(w2e_sb[:], w2e_d[:])
        # shift matrices S[kk, j-1, m] = 1 iff kk == m + j, built on-chip
        # (keeps the single shared DMA engine free for the x loads)
        # s_sb[:, 0, :] is the identity (used for PE psum-accumulation
        # of tap products); s_sb[:, j, :] is the j-shift matrix.
        s_sb = const.tile([128, 7, 118], bf16)
        nc.gpsimd.memset(s_sb[:], 1.0)
        nc.gpsimd.affine_select(
            s_sb[:], s_sb[:], pattern=[[-1, 7], [-1, 118]],
            compare_op=mybir.AluOpType.is_equal, fill=0.0, base=0,
            channel_multiplier=1)

        xTs = []
        for j in range(KK):
            xtj = xpool.tile([128, PR * 256], bf16, name=f"xTs{j}")
            xTs.append(xtj)
        nc.sync.dma_start(xTs[0][0:64], xt_d[0:64])
        nc.scalar.dma_start(xTs[0][64:128], xt_d[64:128])

        # ---- stage 1: t_ext = [relu(W1p @ x + b1p); ones] ----
        t_ext = tpool.tile([Cr + 1, HQ, NCOL], bf16)
        nc.vector.memset(t_ext[Cr:Cr + 1, :, :], 1.0)
        t2 = t_ext[:].rearrange("p a b -> p (a b)")
        with tc.tile_pool(name="psum_t", bufs=2,
                          space=bass.MemorySpace.PSUM) as psum_t:
            NCH = 413  # 4 chunks of 413 = 1652 (<= 512 per psum bank)
            for q in range(4):
                pt = psum_t.tile([Cr, NCH], f32)
                for ch in range(2):
                    nc.tensor.matmul(pt[:], w1t_sb[:, ch, :],
                                     x_cm[:, ch, q * NCH:(q + 1) * NCH],
                                     start=(ch == 0), stop=(ch == 1))
                nc.scalar.activation(
                    t2[0:Cr, q * NCH:(q + 1) * NCH],
                    pt[:], mybir.ActivationFunctionType.Relu,
                    bias=b1p_sb[:], scale=1.0)

        # ---- stage 2: per-rh weight matmul -> wT[(rh, k, g)] ----
        # psum->sbuf copies alternate between ACT and the (still idle) DVE
        # so the copy wall halves.
        wT = wpool.tile([124, HQ, 49, G], bf16)
        with tc.tile_pool(name="psum_w", bufs=3,
                          space=bass.MemorySpace.PSUM) as psum_w:
            for rh in range(HQ):
                # [118, 2, 512]: each gh half starts on a psum bank boundary
                pw = psum_w.tile([NCOL, 2, 512], f32)
                for gh in range(2):
                    nc.tensor.matmul(pw[:, gh, 0:392],
                                     t_ext[:, rh, :],
                                     w2e_sb[:, gh * 392:(gh + 1) * 392],
                                     start=True, stop=True)
                # psum col (gh, k*8+g8) -> wT[p, rh, k, gh*8+g8]
                dst = wT[0:NCOL, rh, :, :].rearrange(
                    "p k (gh g8) -> p k gh g8", gh=2)
                src = pw[:, :, 0:392].rearrange(
                    "p gh (k g8) -> p k gh g8", k=49)
                if rh % 2 == 0:
                    nc.scalar.copy(dst, src)
                else:
                    nc.vector.tensor_copy(dst, src)

        # ---- on-chip j-shifts: xTs[j][m] = xTs[0][m+j] via PE matmul
        # with the shifted identity S[:, j-1, :]; runs on the otherwise
        # idle PE/ACT while the DVE crunches earlier-j taps.
        with tc.tile_pool(name="psum_s", bufs=1,
                          space=bass.MemorySpace.PSUM) as psum_s:
            for j in range(1, KK):
                for c0 in range(0, PR * 256, 1024):
                    ps = psum_s.tile([118, 2, 512], f32)
                    for h in range(2):
                        nc.tensor.matmul(
                            ps[:, h, :], s_sb[:, j, :],
                            xTs[0][:, c0 + h * 512:c0 + (h + 1) * 512],
                            start=True, stop=True)
                    nc.scalar.copy(
                        xTs[j][0:118, c0:c0 + 1024],
                        ps[:].rearrange("p a b -> p (a b)"))

        # ---- stage 3: involution taps ----
        # DVE computes only the 49 products (one 2x tensor_tensor each);
        # the PE accumulates them into a 7-bank f32 PSUM tile via
        # identity matmuls (psum += I @ prod), removing all adds/merges
        # from the DVE stream.
        prodp = ctx.enter_context(tc.tile_pool(name="prod", bufs=8))
        paccp = ctx.enter_context(tc.tile_pool(
            name="psum_acc", bufs=1, space=bass.MemorySpace.PSUM))
        outp = ctx.enter_context(tc.tile_pool(name="outp", bufs=1))
        pacc = paccp.tile([118, FREE3], f32)

        nd = 0
        for j in range(KK):
          for i in range(KK):
            k = i * KK + j
            src = xTs[j][0:118, i * 256:i * 256 + FREE3]
            wsl = wT[0:118, :, k, :].unsqueeze(2).broadcast_to(
                [118, HQ, Cg, G])
            src4 = src.rearrange("p (rh c) -> p rh c", rh=HQ).rearrange(
                "p rh (cg g) -> p rh cg g", g=G)
            dp = prodp.tile([124, FREE3], bf16, tag="dp")
            dp4 = dp[0:118, :].rearrange(
                "p (rh cg g) -> p rh cg g", rh=HQ, g=G)
            nc.vector.tensor_mul(dp4, src4, wsl)
            for c in range(7):
                nc.tensor.matmul(
                    pacc[:, c * 512:(c + 1) * 512], s_sb[0:118, 0, :],
                    dp[0:118, c * 512:(c + 1) * 512],
                    start=(nd == 0), stop=(nd == 48),
                    skip_group_check=True)
            nd += 1

        # psum -> bf16 sbuf -> 3-way DMA (single shared DMA engine)
        ob = outp.tile([118, FREE3], bf16)
        nc.scalar.copy(ob[:], pacc[:])
        nc.sync.dma_start(out_d[0:40], ob[0:40, :])
        nc.scalar.dma_start(out_d[40:79], ob[40:79, :])
        nc.gpsimd.dma_start(out_d[79:118], ob[79:118, :])

    nc.compile()
    return nc


def _prep_host_inputs(inputs, W1, b1, gamma, beta, mean, var, W2, b2):
    """Fold BN into W1/b1; build per-core transposed layouts."""
    scale = gamma / np.sqrt(var + EPS)
    shift = beta - mean * scale
    W1p = W1 * scale[:, None]
    b1p = (b1 * scale + shift).astype(np.float32).reshape(Cr, 1)
    w1t = np.ascontiguousarray(W1p.T.reshape(2, 128, Cr)).astype(BF16)

    # W2e [65, 784]: row 64 = b2; col n = gh*392 + k*8 + g8 for g = gh*8+g8
    w2full = np.concatenate([W2, b2[:, None]], axis=1)  # [784, 65]
    w2g = w2full.reshape(G, 49, Cr + 1)                 # [g, k, cr+1]
    w2e = w2g.reshape(2, 8, 49, Cr + 1).transpose(3, 0, 2, 1)  # [65,2,49,8]
    w2e = np.ascontiguousarray(w2e.reshape(Cr + 1, 784)).astype(BF16)

    xcms, xts = [], []
    for core in range(NCORES):
        b, hf = core // 2, core % 2
        slab = np.zeros((C, HH + 2 * PAD, PW), np.float32)
        r0 = hf * HH - PAD
        r1 = r0 + HH + 2 * PAD
        v0, v1 = max(r0, 0), min(r1, H)
        slab[:, v0 - r0:v1 - r0, PAD:PAD + W] = inputs[b, :, v0:v1, :]
        slab = slab.astype(BF16)  # [256, 34, 62]

        # x_cm [2, 128, (rh:14, ckpw:118)]: pixel col (rh, ck, pw) is the
        # CENTER value slab[c, PAD + ck*14 + rh, pw'] where the stage-1
        # conv is 1x1 at output pixel (ck*14+rh, pw-PAD): center padded
        # coords = (PAD + ck*14 + rh, pw) with pw = PAD+w... but we keep
        # all 62 cols for ck0 (cols pw=56..61 are junk pixels).
        # column ckpw maps to output pixel (14ck+rh, w=pw): center value
        # is slab[c, PAD + 14ck + rh, PAD + pw]; ck0 cols 56..61 are junk
        xcm = np.zeros((C, HQ, NCOL), BF16)
        xcm[:, :, 0:W] = slab[:, PAD:PAD + HQ, PAD:PAD + W]        # ck0
        xcm[:, :, PW:NCOL] = slab[:, PAD + HQ:PAD + HH, PAD:PAD + W]
        xcms.append(np.ascontiguousarray(xcm.reshape(C, NPIX)
                                         .reshape(2, 128, NPIX)))

        # xT [134, (r:20, cg:16, g:16)]: row 62ck+pw, 10 zero pad rows
        sg = slab.reshape(G, Cg, HH + 2 * PAD, PW)  # [g, cg, 34, 62]
        xt = np.zeros((134, PR, Cg, G), BF16)
        for ck in range(2):
            blk = sg[:, :, ck * HQ:ck * HQ + PR, :]  # [g, cg, 20, 62]
            xt[ck * PW:(ck + 1) * PW] = blk.transpose(3, 2, 1, 0)
        xts.append(np.ascontiguousarray(xt.reshape(134, PR * 256)))

    return xcms, xts, w1t, b1p, w2e


def kernel(inputs, W1, b1, gamma, beta, mean, var, W2, b2):
    global LAST_RESULT
    inputs = np.asarray(inputs, np.float32)
    if "nc" not in _CACHE:
        _CACHE["nc"] = _build_nc()
    nc = _CACHE["nc"]

    xcms, xts, w1t, b1p, w2e = _prep_host_inputs(
        inputs, np.asarray(W1, np.float32), np.asarray(b1, np.float32),
        np.asarray(gamma, np.float32), np.asarray(beta, np.float32),
        np.asarray(mean, np.float32), np.asarray(var, np.float32),
        np.asarray(W2, np.float32), np.asarray(b2, np.float32))

    in_maps = [{"xcm": xcms[core], "xt": xts[core], "w1t": w1t,
                "b1p": b1p, "w2e": w2e} for core in range(NCORES)]
    res = run_bass_kernel_spmd(nc, in_maps, list(range(NCORES)), trace=TRACE)
    LAST_RESULT = res

    out = np.empty((B, C, H, W), np.float32)
    for core in range(NCORES):
        b, hf = core // 2, core % 2
        o = res.results[core]["out"].astype(np.float32)  # [118, 3584] bf16
        o6 = o.reshape(NCOL, HQ, Cg, G)       # [ckpw, rh, cg, g]
        slab = np.empty((C, HH, W), np.float32)
        for ck in range(2):
            cols = o6[ck * PW:ck * PW + W]    # [56, 14, 16, 16]
            # -> [c = g*16+cg, rh, w]
            slab[:, ck * HQ:(ck + 1) * HQ, :] = (
                cols.transpose(3, 2, 1, 0).reshape(C, HQ, W))
        out[b, :, hf * HH:(hf + 1) * HH, :] = slab
    return out


# revision 7
# speedup vs baseline: 1.9789x; 1.0089x over previous
"""Involution (B=4, C=256, H=W=56, K=7, G=16, reduction=4) on 8 trn2 NeuronCores.

v2: pixel-major involution with zero-materialization weight broadcast.

Sharding: 8 shards = (batch b in 0..3) x (h-half hf in 0..1); each core
computes a [256, 28, 56] output slab.

Per-core layout (all bf16 in SBUF):
  - Pixel-major transposed input xT[124, (r:20, cg:16, g:16)]: partition
    p = 62*ck + pw where ck splits the 28 output rows into 2x14 and pw is
    the padded column (0..61); r is the padded row within the ck chunk
    (14+2*3=20); channel c = g*16+cg stored g-innermost so that per-group
    weights broadcast via a stride-0 middle AP dim.
  - Stage 1 (PE+ACT): t_ext[65, (rh:14, ckpw:118)] = [relu(W1p@x+b1p); 1]
    from channel-major x_cm; BN folded into W1p/b1p on host.
  - Stage 2 (PE+ACT): per rh: psum[118, 784] = t_ext[:,rh,:]^T @ W2e
    (cols n = gh*392 + k*8 + g8), one ACT copy -> wT[124,(rh,k,g)] bf16.
  - Stage 3 (DVE+GPSIMD): per tap k=(i,j), ONE op over the whole slab:
      acc += xT[j:j+118, i*256 : i*256+3584] * wT[0:118, :, k, :]-bcast
    The weight read uses AP [(rh,784),(cg,0),(g,1)] - stride-0 broadcast,
    verified to run in DVE 2x mode. ~11 taps run on gpsimd instead.
  - Merge accumulators -> f32, DMA out; host un-permutes.
"""

import numpy as np
import ml_dtypes
from contextlib import ExitStack

import concourse.bass as bass
import concourse.bacc as bacc
import concourse.tile as tile
from concourse import mybir
from concourse.bass_utils import run_bass_kernel_spmd

BF16 = ml_dtypes.bfloat16

B, C, H, W = 4, 256, 56, 56
KK, G, PAD = 7, 16, 3
Cr, Cg = 64, 16
EPS = 1e-5
HH = H // 2              # 28 rows per h-half shard
HQ = HH // 2             # 14 rows per sub-chunk (ck)
PW = W + 2 * PAD         # 62 padded width
PR = HQ + 2 * PAD        # 20 padded rows per sub-chunk
NCOL = PW + W            # 118 = ck0 cols 0..61 + ck1 cols 0..55
NPIX = HQ * NCOL         # 1652 stage-1/2 pixel columns
FREE3 = HQ * 256         # 3584 = stage-3 free size
NCORES = 8

# GPSIMD shares SBUF ports with the DVE: running Pool tensor_tensor ops
# concurrently with DVE taps collapses DVE throughput ~4.3x (measured), so
# stage 3 runs entirely on the DVE.
N_DVE_ACC = 4

_CACHE = {}
TRACE = False
LAST_RESULT = None


def _build_nc():
    nc = bacc.Bacc("TRN2", target_bir_lowering=False, debug=False,
                   num_devices=NCORES)

    f32 = mybir.dt.float32
    bf16 = mybir.dt.bfloat16

    xcm_d = nc.declare_dram_parameter("xcm", [2, 128, NPIX], bf16,
                                      isOutput=False)
    # 134 rows: 124 (ck,pw) rows + 10 zero rows so each shifted load
    # xTs[j] = xt[j : j+128] stays in range for j <= 6
    xt_d = nc.declare_dram_parameter("xt", [134, PR * 256], bf16,
                                     isOutput=False)
    w1t_d = nc.declare_dram_parameter("w1t", [2, 128, Cr], bf16,
                                      isOutput=False)
    b1p_d = nc.declare_dram_parameter("b1p", [Cr, 1], f32, isOutput=False)
    w2e_d = nc.declare_dram_parameter("w2e", [Cr + 1, 784], bf16,
                                      isOutput=False)
    out_d = nc.declare_dram_parameter("out", [118, FREE3], bf16,
                                      isOutput=True)

    with tile.TileContext(nc) as tc, ExitStack() as ctx:
        const = ctx.enter_context(tc.tile_pool(name="const", bufs=1))
        xpool = ctx.enter_context(tc.tile_pool(name="x", bufs=1))
        tpool = ctx.enter_context(tc.tile_pool(name="t", bufs=1))
        wpool = ctx.enter_context(tc.tile_pool(name="w", bufs=1))

        # Only xTs[0] is loaded from DRAM (split across both HWDGE
        # queues); the 6 shifted copies are produced on-chip by the PE
        # (matmul with a shifted identity = partition shift), which keeps
        # the DMA engines quiet during the DVE tap window.
        x_cm = xpool.tile([128, 2, NPIX], bf16)
        nc.sync.dma_start(x_cm[:, 0, :], xcm_d[0])
        nc.scalar.dma_start(x_cm[:, 1, :], xcm_d[1])

        w1t_sb = const.tile([128, 2, Cr], bf16)
        for ch in range(2):
            nc.sync.dma_start(w1t_sb[:, ch, :], w1t_d[ch])
        b1p_sb = const.tile([Cr, 1], f32)
        nc.sync.dma_start(b1p_sb[:], b1p_d[:])
        w2e_sb = const.tile([Cr + 1, 784], bf16)
        nc.scalar.dma_start(w2e_sb[:], w2e_d[:])
        # shift matrices S[kk, j-1, m] = 1 iff kk == m + j, built on-chip
        # (keeps the single shared DMA engine free for the x loads)
        # s_sb[:, 0, :] is the identity (used for PE psum-accumulation
        # of tap products); s_sb[:, j, :] is the j-shift matrix.
        s_sb = const.tile([128, 7, 118], bf16)
        nc.gpsimd.memset(s_sb[:], 1.0)
        nc.gpsimd.affine_select(
            s_sb[:], s_sb[:], pattern=[[-1, 7], [-1, 118]],
            compare_op=mybir.AluOpType.is_equal, fill=0.0, base=0,
            channel_multiplier=1)

        xTs = []
        for j in range(KK):
            xtj = xpool.tile([128, PR * 256], bf16, name=f"xTs{j}")
            xTs.append(xtj)
        nc.sync.dma_start(xTs[0][0:64], xt_d[0:64])
        nc.scalar.dma_start(xTs[0][64:128], xt_d[64:128])

        # ---- stage 1: t_ext = [relu(W1p @ x + b1p); ones] ----
        t_ext = tpool.tile([Cr + 1, HQ, NCOL], bf16)
        nc.vector.memset(t_ext[Cr:Cr + 1, :, :], 1.0)
        t2 = t_ext[:].rearrange("p a b -> p (a b)")
        with tc.tile_pool(name="psum_t", bufs=2,
                          space=bass.MemorySpace.PSUM) as psum_t:
            NCH = 413  # 4 chunks of 413 = 1652 (<= 512 per psum bank)
            for q in range(4):
                pt = psum_t.tile([Cr, NCH], f32)
                for ch in range(2):
                    nc.tensor.matmul(pt[:], w1t_sb[:, ch, :],
                                     x_cm[:, ch, q * NCH:(q + 1) * NCH],
                                     start=(ch == 0), stop=(ch == 1))
                nc.scalar.activation(
                    t2[0:Cr, q * NCH:(q + 1) * NCH],
                    pt[:], mybir.ActivationFunctionType.Relu,
                    bias=b1p_sb[:], scale=1.0)

        # ---- stage 2: per-rh weight matmul -> wT[(rh, k, g)] ----
        # psum->sbuf copies alternate between ACT and the (still idle) DVE
        # so the copy wall halves.
        wT = wpool.tile([124, HQ, 49, G], bf16)
        with tc.tile_pool(name="psum_w", bufs=3,
                          space=bass.MemorySpace.PSUM) as psum_w:
            for rh in range(HQ):
                # [118, 2, 512]: each gh half starts on a psum bank boundary
                pw = psum_w.tile([NCOL, 2, 512], f32)
                for gh in range(2):
                    nc.tensor.matmul(pw[:, gh, 0:392],
                                     t_ext[:, rh, :],
                                     w2e_sb[:, gh * 392:(gh + 1) * 392],
                                     start=True, stop=True)
                # psum col (gh, k*8+g8) -> wT[p, rh, k, gh*8+g8]
                dst = wT[0:NCOL, rh, :, :].rearrange(
                    "p k (gh g8) -> p k gh g8", gh=2)
                src = pw[:, :, 0:392].rearrange(
                    "p gh (k g8) -> p k gh g8", k=49)
                if rh % 2 == 0:
                    nc.scalar.copy(dst, src)
                else:
                    nc.vector.tensor_copy(dst, src)

        # ---- on-chip j-shifts: xTs[j][m] = xTs[0][m+j] via PE matmul
        # with the shifted identity S[:, j-1, :]; runs on the otherwise
        # idle PE/ACT while the DVE crunches earlier-j taps.
        with tc.tile_pool(name="psum_s", bufs=3,
                          space=bass.MemorySpace.PSUM) as psum_s:
            for j in range(1, KK):
                for c0 in range(0, PR * 256, 1024):
                    ps = psum_s.tile([118, 2, 512], f32)
                    for h in range(2):
                        nc.tensor.matmul(
                            ps[:, h, :], s_sb[:, j, :],
                            xTs[0][:, c0 + h * 512:c0 + (h + 1) * 512],
                            start=True, stop=True)
                    nc.scalar.copy(
                        xTs[j][0:118, c0:c0 + 1024],
                        ps[:].rearrange("p a b -> p (a b)"))

        # ---- stage 3: involution taps ----
        # DVE computes only the 49 products (one 2x tensor_tensor each);
        # the PE accumulates them into a 7-bank f32 PSUM tile via
        # identity matmuls (psum += I @ prod), removing all adds/merges
        # from the DVE stream.
        prodp = ctx.enter_context(tc.tile_pool(name="prod", bufs=8))
        paccp = ctx.enter_context(tc.tile_pool(
            name="psum_acc", bufs=1, space=bass.MemorySpace.PSUM))
        outp = ctx.enter_context(tc.tile_pool(name="outp", bufs=1))
        pacc = paccp.tile([118, FREE3], f32)

        nd = 0
        for j in range(KK):
          for i in range(KK):
            k = i * KK + j
            src = xTs[j][0:118, i * 256:i * 256 + FREE3]
            wsl = wT[0:118, :, k, :].unsqueeze(2).broadcast_to(
                [118, HQ, Cg, G])
            src4 = src.rearrange("p (rh c) -> p rh c", rh=HQ).rearrange(
                "p rh (cg g) -> p rh cg g", g=G)
            dp = prodp.tile([124, FREE3], bf16, tag="dp")
            dp4 = dp[0:118, :].rearrange(
                "p (rh cg g) -> p rh cg g", rh=HQ, g=G)
            nc.vector.tensor_mul(dp4, src4, wsl)
            for c in range(7):
                nc.tensor.matmul(
                    pacc[:, c * 512:(c + 1) * 512], s_sb[0:118, 0, :],
                    dp[0:118, c * 512:(c + 1) * 512],
                    start=(nd == 0), stop=(nd == 48),
                    skip_group_check=True)
            nd += 1

        # psum -> bf16 sbuf -> 3-way DMA (single shared DMA engine)
        ob = outp.tile([118, FREE3], bf16)
        nc.scalar.copy(ob[:], pacc[:])
        nc.sync.dma_start(out_d[0:40], ob[0:40, :])
        nc.scalar.dma_start(out_d[40:79], ob[40:79, :])
        nc.gpsimd.dma_start(out_d[79:118], ob[79:118, :])

    nc.compile()
    return nc


def _prep_host_inputs(inputs, W1, b1, gamma, beta, mean, var, W2, b2):
    """Fold BN into W1/b1; build per-core transposed layouts."""
    scale = gamma / np.sqrt(var + EPS)
    shift = beta - mean * scale
    W1p = W1 * scale[:, None]
    b1p = (b1 * scale + shift).astype(np.float32).reshape(Cr, 1)
    w1t = np.ascontiguousarray(W1p.T.reshape(2, 128, Cr)).astype(BF16)

    # W2e [65, 784]: row 64 = b2; col n = gh*392 + k*8 + g8 for g = gh*8+g8
    w2full = np.concatenate([W2, b2[:, None]], axis=1)  # [784, 65]
    w2g = w2full.reshape(G, 49, Cr + 1)                 # [g, k, cr+1]
    w2e = w2g.reshape(2, 8, 49, Cr + 1).transpose(3, 0, 2, 1)  # [65,2,49,8]
    w2e = np.ascontiguousarray(w2e.reshape(Cr + 1, 784)).astype(BF16)

    xcms, xts = [], []
    for core in range(NCORES):
        b, hf = core // 2, core % 2
        slab = np.zeros((C, HH + 2 * PAD, PW), np.float32)
        r0 = hf * HH - PAD
        r1 = r0 + HH + 2 * PAD
        v0, v1 = max(r0, 0), min(r1, H)
        slab[:, v0 - r0:v1 - r0, PAD:PAD + W] = inputs[b, :, v0:v1, :]
        slab = slab.astype(BF16)  # [256, 34, 62]

        # x_cm [2, 128, (rh:14, ckpw:118)]: pixel col (rh, ck, pw) is the
        # CENTER value slab[c, PAD + ck*14 + rh, pw'] where the stage-1
        # conv is 1x1 at output pixel (ck*14+rh, pw-PAD): center padded
        # coords = (PAD + ck*14 + rh, pw) with pw = PAD+w... but we keep
        # all 62 cols for ck0 (cols pw=56..61 are junk pixels).
        # column ckpw maps to output pixel (14ck+rh, w=pw): center value
        # is slab[c, PAD + 14ck + rh, PAD + pw]; ck0 cols 56..61 are junk
        xcm = np.zeros((C, HQ, NCOL), BF16)
        xcm[:, :, 0:W] = slab[:, PAD:PAD + HQ, PAD:PAD + W]        # ck0
        xcm[:, :, PW:NCOL] = slab[:, PAD + HQ:PAD + HH, PAD:PAD + W]
        xcms.append(np.ascontiguousarray(xcm.reshape(C, NPIX)
                                         .reshape(2, 128, NPIX)))

        # xT [134, (r:20, cg:16, g:16)]: row 62ck+pw, 10 zero pad rows
        sg = slab.reshape(G, Cg, HH + 2 * PAD, PW)  # [g, cg, 34, 62]
        xt = np.zeros((134, PR, Cg, G), BF16)
        for ck in range(2):
            blk = sg[:, :, ck * HQ:ck * HQ + PR, :]  # [g, cg, 20, 62]
            xt[ck * PW:(ck + 1) * PW] = blk.transpose(3, 2, 1, 0)
        xts.append(np.ascontiguousarray(xt.reshape(134, PR * 256)))

    return xcms, xts, w1t, b1p, w2e


def kernel(inputs, W1, b1, gamma, beta, mean, var, W2, b2):
    global LAST_RESULT
    inputs = np.asarray(inputs, np.float32)
    if "nc" not in _CACHE:
        _CACHE["nc"] = _build_nc()
    nc = _CACHE["nc"]

    xcms, xts, w1t, b1p, w2e = _prep_host_inputs(
        inputs, np.asarray(W1, np.float32), np.asarray(b1, np.float32),
        np.asarray(gamma, np.float32), np.asarray(beta, np.float32),
        np.asarray(mean, np.float32), np.asarray(var, np.float32),
        np.asarray(W2, np.float32), np.asarray(b2, np.float32))

    in_maps = [{"xcm": xcms[core], "xt": xts[core], "w1t": w1t,
                "b1p": b1p, "w2e": w2e} for core in range(NCORES)]
    res = run_bass_kernel_spmd(nc, in_maps, list(range(NCORES)), trace=TRACE)
    LAST_RESULT = res

    out = np.empty((B, C, H, W), np.float32)
    for core in range(NCORES):
        b, hf = core // 2, core % 2
        o = res.results[core]["out"].astype(np.float32)  # [118, 3584] bf16
        o6 = o.reshape(NCOL, HQ, Cg, G)       # [ckpw, rh, cg, g]
        slab = np.empty((C, HH, W), np.float32)
        for ck in range(2):
            cols = o6[ck * PW:ck * PW + W]    # [56, 14, 16, 16]
            # -> [c = g*16+cg, rh, w]
            slab[:, ck * HQ:(ck + 1) * HQ, :] = (
                cols.transpose(3, 2, 1, 0).reshape(C, HQ, W))
        out[b, :, hf * HH:(hf + 1) * HH, :] = slab
    return out


# revision 8
# speedup vs baseline: 2.0584x; 1.0402x over previous
"""Involution (B=4, C=256, H=W=56, K=7, G=16, reduction=4) on 8 trn2 NeuronCores.

v2: pixel-major involution with zero-materialization weight broadcast.

Sharding: 8 shards = (batch b in 0..3) x (h-half hf in 0..1); each core
computes a [256, 28, 56] output slab.

Per-core layout (all bf16 in SBUF):
  - Pixel-major transposed input xT[124, (r:20, cg:16, g:16)]: partition
    p = 62*ck + pw where ck splits the 28 output rows into 2x14 and pw is
    the padded column (0..61); r is the padded row within the ck chunk
    (14+2*3=20); channel c = g*16+cg stored g-innermost so that per-group
    weights broadcast via a stride-0 middle AP dim.
  - Stage 1 (PE+ACT): t_ext[65, (rh:14, ckpw:118)] = [relu(W1p@x+b1p); 1]
    from channel-major x_cm; BN folded into W1p/b1p on host.
  - Stage 2 (PE+ACT): per rh: psum[118, 784] = t_ext[:,rh,:]^T @ W2e
    (cols n = gh*392 + k*8 + g8), one ACT copy -> wT[124,(rh,k,g)] bf16.
  - Stage 3 (DVE+GPSIMD): per tap k=(i,j), ONE op over the whole slab:
      acc += xT[j:j+118, i*256 : i*256+3584] * wT[0:118, :, k, :]-bcast
    The weight read uses AP [(rh,784),(cg,0),(g,1)] - stride-0 broadcast,
    verified to run in DVE 2x mode. ~11 taps run on gpsimd instead.
  - Merge accumulators -> f32, DMA out; host un-permutes.
"""

import numpy as np
import ml_dtypes
from contextlib import ExitStack

import concourse.bass as bass
import concourse.bacc as bacc
import concourse.tile as tile
from concourse import mybir
from concourse.bass_utils import run_bass_kernel_spmd

BF16 = ml_dtypes.bfloat16

B, C, H, W = 4, 256, 56, 56
KK, G, PAD = 7, 16, 3
Cr, Cg = 64, 16
EPS = 1e-5
HH = H // 2              # 28 rows per h-half shard
HQ = HH // 2             # 14 rows per sub-chunk (ck)
PW = W + 2 * PAD         # 62 padded width
PR = HQ + 2 * PAD        # 20 padded rows per sub-chunk
NCOL = PW + W            # 118 = ck0 cols 0..61 + ck1 cols 0..55
NPIX = HQ * NCOL         # 1652 stage-1/2 pixel columns
FREE3 = HQ * 256         # 3584 = stage-3 free size
NCORES = 8

# GPSIMD shares SBUF ports with the DVE: running Pool tensor_tensor ops
# concurrently with DVE taps collapses DVE throughput ~4.3x (measured), so
# stage 3 runs entirely on the DVE.
N_DVE_ACC = 4

_CACHE = {}
TRACE = False
LAST_RESULT = None


def _build_nc():
    nc = bacc.Bacc("TRN2", target_bir_lowering=False, debug=False,
                   num_devices=NCORES)

    f32 = mybir.dt.float32
    bf16 = mybir.dt.bfloat16

    xcm_d = nc.declare_dram_parameter("xcm", [2, 128, NPIX], bf16,
                                      isOutput=False)
    # 134 rows: 124 (ck,pw) rows + 10 zero rows so each shifted load
    # xTs[j] = xt[j : j+128] stays in range for j <= 6
    xt_d = nc.declare_dram_parameter("xt", [134, PR * 256], bf16,
                                     isOutput=False)
    w1t_d = nc.declare_dram_parameter("w1t", [2, 128, Cr], bf16,
                                      isOutput=False)
    b1p_d = nc.declare_dram_parameter("b1p", [Cr, 1], f32, isOutput=False)
    w2e_d = nc.declare_dram_parameter("w2e", [Cr + 1, 784], bf16,
                                      isOutput=False)
    out_d = nc.declare_dram_parameter("out", [118, FREE3], bf16,
                                      isOutput=True)

    with tile.TileContext(nc) as tc, ExitStack() as ctx:
        const = ctx.enter_context(tc.tile_pool(name="const", bufs=1))
        xpool = ctx.enter_context(tc.tile_pool(name="x", bufs=1))
        tpool = ctx.enter_context(tc.tile_pool(name="t", bufs=1))
        wpool = ctx.enter_context(tc.tile_pool(name="w", bufs=1))

        # Only xTs[0] is loaded from DRAM (split across both HWDGE
        # queues); the 6 shifted copies are produced on-chip by the PE
        # (matmul with a shifted identity = partition shift), which keeps
        # the DMA engines quiet during the DVE tap window.
        x_cm = xpool.tile([128, 2, NPIX], bf16)
        nc.sync.dma_start(x_cm[:, 0, :], xcm_d[0])
        nc.scalar.dma_start(x_cm[:, 1, :], xcm_d[1])

        w1t_sb = const.tile([128, 2, Cr], bf16)
        for ch in range(2):
            nc.sync.dma_start(w1t_sb[:, ch, :], w1t_d[ch])
        b1p_sb = const.tile([Cr, 1], f32)
        nc.sync.dma_start(b1p_sb[:], b1p_d[:])
        w2e_sb = const.tile([Cr + 1, 784], bf16)
        nc.scalar.dma_start(w2e_sb[:], w2e_d[:])
        # shift matrices S[kk, j-1, m] = 1 iff kk == m + j, built on-chip
        # (keeps the single shared DMA engine free for the x loads)
        # s_sb[:, 0, :] is the identity (used for PE psum-accumulation
        # of tap products); s_sb[:, j, :] is the j-shift matrix.
        s_sb = const.tile([128, 7, 118], bf16)
        nc.gpsimd.memset(s_sb[:], 1.0)
        nc.gpsimd.affine_select(
            s_sb[:], s_sb[:], pattern=[[-1, 7], [-1, 118]],
            compare_op=mybir.AluOpType.is_equal, fill=0.0, base=0,
            channel_multiplier=1)

        xTs = []
        for j in range(KK):
            xtj = xpool.tile([128, PR * 256], bf16, name=f"xTs{j}")
            xTs.append(xtj)
        nc.sync.dma_start(xTs[0][0:64], xt_d[0:64])
        nc.scalar.dma_start(xTs[0][64:128], xt_d[64:128])

        # ---- stage 1: t_ext = [relu(W1p @ x + b1p); ones] ----
        t_ext = tpool.tile([Cr + 1, HQ, NCOL], bf16)
        nc.vector.memset(t_ext[Cr:Cr + 1, :, :], 1.0)
        t2 = t_ext[:].rearrange("p a b -> p (a b)")
        with tc.tile_pool(name="psum_t", bufs=2,
                          space=bass.MemorySpace.PSUM) as psum_t:
            NCH = 413  # 4 chunks of 413 = 1652 (<= 512 per psum bank)
            for q in range(4):
                pt = psum_t.tile([Cr, NCH], f32)
                for ch in range(2):
                    nc.tensor.matmul(pt[:], w1t_sb[:, ch, :],
                                     x_cm[:, ch, q * NCH:(q + 1) * NCH],
                                     start=(ch == 0), stop=(ch == 1))
                nc.scalar.activation(
                    t2[0:Cr, q * NCH:(q + 1) * NCH],
                    pt[:], mybir.ActivationFunctionType.Relu,
                    bias=b1p_sb[:], scale=1.0)

        # ---- stage 2: per-rh weight matmul -> wT[(rh, k, g)] ----
        # psum->sbuf copies alternate between ACT and the (still idle) DVE
        # so the copy wall halves.
        wT = wpool.tile([124, HQ, 49, G], bf16)
        with tc.tile_pool(name="psum_w", bufs=3,
                          space=bass.MemorySpace.PSUM) as psum_w:
            for rh in range(HQ):
                # [118, 2, 512]: each gh half starts on a psum bank boundary
                pw = psum_w.tile([NCOL, 2, 512], f32)
                for gh in range(2):
                    nc.tensor.matmul(pw[:, gh, 0:392],
                                     t_ext[:, rh, :],
                                     w2e_sb[:, gh * 392:(gh + 1) * 392],
                                     start=True, stop=True)
                # psum col (gh, k*8+g8) -> wT[p, rh, k, gh*8+g8]
                dst = wT[0:NCOL, rh, :, :].rearrange(
                    "p k (gh g8) -> p k gh g8", gh=2)
                src = pw[:, :, 0:392].rearrange(
                    "p gh (k g8) -> p k gh g8", k=49)
                if rh % 2 == 0:
                    nc.scalar.copy(dst, src)
                else:
                    nc.vector.tensor_copy(dst, src)

        # ---- on-chip j-shifts: xTs[j][m] = xTs[0][m+j] via PE matmul
        # with the shifted identity S[:, j-1, :]; runs on the otherwise
        # idle PE/ACT while the DVE crunches earlier-j taps.
        with tc.tile_pool(name="psum_s", bufs=2,
                          space=bass.MemorySpace.PSUM) as psum_s:
            for j in range(1, KK):
                for c0 in range(0, PR * 256, 2048):
                    nch = min(2048, PR * 256 - c0)
                    ps = psum_s.tile([118, 4, 512], f32)
                    for h in range(nch // 512):
                        nc.tensor.matmul(
                            ps[:, h, :], s_sb[:, j, :],
                            xTs[0][:, c0 + h * 512:c0 + (h + 1) * 512],
                            start=True, stop=True)
                    nc.scalar.copy(
                        xTs[j][0:118, c0:c0 + nch],
                        ps[:, 0:nch // 512, :].rearrange(
                            "p a b -> p (a b)"))

        # ---- stage 3: involution taps ----
        # DVE computes only the 49 products (one 2x tensor_tensor each);
        # the PE accumulates them into a 7-bank f32 PSUM tile via
        # identity matmuls (psum += I @ prod), removing all adds/merges
        # from the DVE stream.
        prodp = ctx.enter_context(tc.tile_pool(name="prod", bufs=8))
        paccp = ctx.enter_context(tc.tile_pool(
            name="psum_acc", bufs=1, space=bass.MemorySpace.PSUM))
        outp = ctx.enter_context(tc.tile_pool(name="outp", bufs=1))
        pacc = paccp.tile([118, FREE3], f32)

        nd = 0
        for j in range(KK):
          for i in range(KK):
            k = i * KK + j
            src = xTs[j][0:118, i * 256:i * 256 + FREE3]
            wsl = wT[0:118, :, k, :].unsqueeze(2).broadcast_to(
                [118, HQ, Cg, G])
            src4 = src.rearrange("p (rh c) -> p rh c", rh=HQ).rearrange(
                "p rh (cg g) -> p rh cg g", g=G)
            dp = prodp.tile([124, FREE3], bf16, tag="dp")
            dp4 = dp[0:118, :].rearrange(
                "p (rh cg g) -> p rh cg g", rh=HQ, g=G)
            nc.vector.tensor_mul(dp4, src4, wsl)
            for c in range(7):
                nc.tensor.matmul(
                    pacc[:, c * 512:(c + 1) * 512], s_sb[0:118, 0, :],
                    dp[0:118, c * 512:(c + 1) * 512],
                    start=(nd == 0), stop=(nd == 48),
                    skip_group_check=True)
            nd += 1

        # psum -> bf16 sbuf -> 3-way DMA (single shared DMA engine)
        ob = outp.tile([118, FREE3], bf16)
        nc.scalar.copy(ob[:], pacc[:])
        nc.sync.dma_start(out_d[0:40], ob[0:40, :])
        nc.scalar.dma_start(out_d[40:79], ob[40:79, :])
        nc.gpsimd.dma_start(out_d[79:118], ob[79:118, :])

    nc.compile()
    return nc


def _prep_host_inputs(inputs, W1, b1, gamma, beta, mean, var, W2, b2):
    """Fold BN into W1/b1; build per-core transposed layouts."""
    scale = gamma / np.sqrt(var + EPS)
    shift = beta - mean * scale
    W1p = W1 * scale[:, None]
    b1p = (b1 * scale + shift).astype(np.float32).reshape(Cr, 1)
    w1t = np.ascontiguousarray(W1p.T.reshape(2, 128, Cr)).astype(BF16)

    # W2e [65, 784]: row 64 = b2; col n = gh*392 + k*8 + g8 for g = gh*8+g8
    w2full = np.concatenate([W2, b2[:, None]], axis=1)  # [784, 65]
    w2g = w2full.reshape(G, 49, Cr + 1)                 # [g, k, cr+1]
    w2e = w2g.reshape(2, 8, 49, Cr + 1).transpose(3, 0, 2, 1)  # [65,2,49,8]
    w2e = np.ascontiguousarray(w2e.reshape(Cr + 1, 784)).astype(BF16)

    xcms, xts = [], []
    for core in range(NCORES):
        b, hf = core // 2, core % 2
        slab = np.zeros((C, HH + 2 * PAD, PW), np.float32)
        r0 = hf * HH - PAD
        r1 = r0 + HH + 2 * PAD
        v0, v1 = max(r0, 0), min(r1, H)
        slab[:, v0 - r0:v1 - r0, PAD:PAD + W] = inputs[b, :, v0:v1, :]
        slab = slab.astype(BF16)  # [256, 34, 62]

        # x_cm [2, 128, (rh:14, ckpw:118)]: pixel col (rh, ck, pw) is the
        # CENTER value slab[c, PAD + ck*14 + rh, pw'] where the stage-1
        # conv is 1x1 at output pixel (ck*14+rh, pw-PAD): center padded
        # coords = (PAD + ck*14 + rh, pw) with pw = PAD+w... but we keep
        # all 62 cols for ck0 (cols pw=56..61 are junk pixels).
        # column ckpw maps to output pixel (14ck+rh, w=pw): center value
        # is slab[c, PAD + 14ck + rh, PAD + pw]; ck0 cols 56..61 are junk
        xcm = np.zeros((C, HQ, NCOL), BF16)
        xcm[:, :, 0:W] = slab[:, PAD:PAD + HQ, PAD:PAD + W]        # ck0
        xcm[:, :, PW:NCOL] = slab[:, PAD + HQ:PAD + HH, PAD:PAD + W]
        xcms.append(np.ascontiguousarray(xcm.reshape(C, NPIX)
                                         .reshape(2, 128, NPIX)))

        # xT [134, (r:20, cg:16, g:16)]: row 62ck+pw, 10 zero pad rows
        sg = slab.reshape(G, Cg, HH + 2 * PAD, PW)  # [g, cg, 34, 62]
        xt = np.zeros((134, PR, Cg, G), BF16)
        for ck in range(2):
            blk = sg[:, :, ck * HQ:ck * HQ + PR, :]  # [g, cg, 20, 62]
            xt[ck * PW:(ck + 1) * PW] = blk.transpose(3, 2, 1, 0)
        xts.append(np.ascontiguousarray(xt.reshape(134, PR * 256)))

    return xcms, xts, w1t, b1p, w2e


def kernel(inputs, W1, b1, gamma, beta, mean, var, W2, b2):
    global LAST_RESULT
    inputs = np.asarray(inputs, np.float32)
    if "nc" not in _CACHE:
        _CACHE["nc"] = _build_nc()
    nc = _CACHE["nc"]

    xcms, xts, w1t, b1p, w2e = _prep_host_inputs(
        inputs, np.asarray(W1, np.float32), np.asarray(b1, np.float32),
        np.asarray(gamma, np.float32), np.asarray(beta, np.float32),
        np.asarray(mean, np.float32), np.asarray(var, np.float32),
        np.asarray(W2, np.float32), np.asarray(b2, np.float32))

    in_maps = [{"xcm": xcms[core], "xt": xts[core], "w1t": w1t,
                "b1p": b1p, "w2e": w2e} for core in range(NCORES)]
    res = run_bass_kernel_spmd(nc, in_maps, list(range(NCORES)), trace=TRACE)
    LAST_RESULT = res

    out = np.empty((B, C, H, W), np.float32)
    for core in range(NCORES):
        b, hf = core // 2, core % 2
        o = res.results[core]["out"].astype(np.float32)  # [118, 3584] bf16
        o6 = o.reshape(NCOL, HQ, Cg, G)       # [ckpw, rh, cg, g]
        slab = np.empty((C, HH, W), np.float32)
        for ck in range(2):
            cols = o6[ck * PW:ck * PW + W]    # [56, 14, 16, 16]
            # -> [c = g*16+cg, rh, w]
            slab[:, ck * HQ:(ck + 1) * HQ, :] = (
                cols.transpose(3, 2, 1, 0).reshape(C, HQ, W))
        out[b, :, hf * HH:(hf + 1) * HH, :] = slab
    return out


# revision 9
# speedup vs baseline: 2.0706x; 1.0059x over previous
"""Involution (B=4, C=256, H=W=56, K=7, G=16, reduction=4) on 8 trn2 NeuronCores.

v2: pixel-major involution with zero-materialization weight broadcast.

Sharding: 8 shards = (batch b in 0..3) x (h-half hf in 0..1); each core
computes a [256, 28, 56] output slab.

Per-core layout (all bf16 in SBUF):
  - Pixel-major transposed input xT[124, (r:20, cg:16, g:16)]: partition
    p = 62*ck + pw where ck splits the 28 output rows into 2x14 and pw is
    the padded column (0..61); r is the padded row within the ck chunk
    (14+2*3=20); channel c = g*16+cg stored g-innermost so that per-group
    weights broadcast via a stride-0 middle AP dim.
  - Stage 1 (PE+ACT): t_ext[65, (rh:14, ckpw:118)] = [relu(W1p@x+b1p); 1]
    from channel-major x_cm; BN folded into W1p/b1p on host.
  - Stage 2 (PE+ACT): per rh: psum[118, 784] = t_ext[:,rh,:]^T @ W2e
    (cols n = gh*392 + k*8 + g8), one ACT copy -> wT[124,(rh,k,g)] bf16.
  - Stage 3 (DVE+GPSIMD): per tap k=(i,j), ONE op over the whole slab:
      acc += xT[j:j+118, i*256 : i*256+3584] * wT[0:118, :, k, :]-bcast
    The weight read uses AP [(rh,784),(cg,0),(g,1)] - stride-0 broadcast,
    verified to run in DVE 2x mode. ~11 taps run on gpsimd instead.
  - Merge accumulators -> f32, DMA out; host un-permutes.
"""

import numpy as np
import ml_dtypes
from contextlib import ExitStack

import concourse.bass as bass
import concourse.bacc as bacc
import concourse.tile as tile
from concourse import mybir
from concourse.bass_utils import run_bass_kernel_spmd

BF16 = ml_dtypes.bfloat16

B, C, H, W = 4, 256, 56, 56
KK, G, PAD = 7, 16, 3
Cr, Cg = 64, 16
EPS = 1e-5
HH = H // 2              # 28 rows per h-half shard
HQ = HH // 2             # 14 rows per sub-chunk (ck)
PW = W + 2 * PAD         # 62 padded width
PR = HQ + 2 * PAD        # 20 padded rows per sub-chunk
NCOL = PW + W            # 118 = ck0 cols 0..61 + ck1 cols 0..55
NPIX = HQ * NCOL         # 1652 stage-1/2 pixel columns
FREE3 = HQ * 256         # 3584 = stage-3 free size
NCORES = 8

# GPSIMD shares SBUF ports with the DVE: running Pool tensor_tensor ops
# concurrently with DVE taps collapses DVE throughput ~4.3x (measured), so
# stage 3 runs entirely on the DVE.
N_DVE_ACC = 4

_CACHE = {}
TRACE = False
LAST_RESULT = None


def _build_nc():
    nc = bacc.Bacc("TRN2", target_bir_lowering=False, debug=False,
                   num_devices=NCORES)

    f32 = mybir.dt.float32
    bf16 = mybir.dt.bfloat16

    xcm_d = nc.declare_dram_parameter("xcm", [2, 128, NPIX], bf16,
                                      isOutput=False)
    # 134 rows: 124 (ck,pw) rows + 10 zero rows so each shifted load
    # xTs[j] = xt[j : j+128] stays in range for j <= 6
    xt_d = nc.declare_dram_parameter("xt", [134, PR * 256], bf16,
                                     isOutput=False)
    w1t_d = nc.declare_dram_parameter("w1t", [2, 128, Cr], bf16,
                                      isOutput=False)
    b1p_d = nc.declare_dram_parameter("b1p", [Cr, 1], f32, isOutput=False)
    w2e_d = nc.declare_dram_parameter("w2e", [Cr + 1, 784], bf16,
                                      isOutput=False)
    out_d = nc.declare_dram_parameter("out", [118, FREE3], bf16,
                                      isOutput=True)

    with tile.TileContext(nc) as tc, ExitStack() as ctx:
        const = ctx.enter_context(tc.tile_pool(name="const", bufs=1))
        xpool = ctx.enter_context(tc.tile_pool(name="x", bufs=1))
        tpool = ctx.enter_context(tc.tile_pool(name="t", bufs=1))
        wpool = ctx.enter_context(tc.tile_pool(name="w", bufs=1))

        # Only xTs[0] is loaded from DRAM (split across both HWDGE
        # queues); the 6 shifted copies are produced on-chip by the PE
        # (matmul with a shifted identity = partition shift), which keeps
        # the DMA engines quiet during the DVE tap window.
        x_cm = xpool.tile([128, 2, NPIX], bf16)
        nc.sync.dma_start(x_cm[:, 0, :], xcm_d[0])
        nc.scalar.dma_start(x_cm[:, 1, :], xcm_d[1])

        w1t_sb = const.tile([128, 2, Cr], bf16)
        for ch in range(2):
            nc.sync.dma_start(w1t_sb[:, ch, :], w1t_d[ch])
        b1p_sb = const.tile([Cr, 1], f32)
        nc.sync.dma_start(b1p_sb[:], b1p_d[:])
        w2e_sb = const.tile([Cr + 1, 784], bf16)
        nc.scalar.dma_start(w2e_sb[:], w2e_d[:])
        # shift matrices S[kk, j-1, m] = 1 iff kk == m + j, built on-chip
        # (keeps the single shared DMA engine free for the x loads)
        # s_sb[:, 0, :] is the identity (used for PE psum-accumulation
        # of tap products); s_sb[:, j, :] is the j-shift matrix.
        s_sb = const.tile([128, 7, 118], bf16)
        nc.gpsimd.memset(s_sb[:], 1.0)
        nc.gpsimd.affine_select(
            s_sb[:], s_sb[:], pattern=[[-1, 7], [-1, 118]],
            compare_op=mybir.AluOpType.is_equal, fill=0.0, base=0,
            channel_multiplier=1)

        xTs = []
        for j in range(KK):
            xtj = xpool.tile([128, PR * 256], bf16, name=f"xTs{j}")
            xTs.append(xtj)
        nc.sync.dma_start(xTs[0][0:64], xt_d[0:64])
        nc.scalar.dma_start(xTs[0][64:128], xt_d[64:128])

        # ---- stage 1: t_ext = [relu(W1p @ x + b1p); ones] ----
        t_ext = tpool.tile([Cr + 1, HQ, NCOL], bf16)
        nc.vector.memset(t_ext[Cr:Cr + 1, :, :], 1.0)
        t2 = t_ext[:].rearrange("p a b -> p (a b)")
        with tc.tile_pool(name="psum_t", bufs=2,
                          space=bass.MemorySpace.PSUM) as psum_t:
            NCH = 413  # 4 chunks of 413 = 1652 (<= 512 per psum bank)
            for q in range(4):
                pt = psum_t.tile([Cr, NCH], f32)
                for ch in range(2):
                    nc.tensor.matmul(pt[:], w1t_sb[:, ch, :],
                                     x_cm[:, ch, q * NCH:(q + 1) * NCH],
                                     start=(ch == 0), stop=(ch == 1))
                nc.scalar.activation(
                    t2[0:Cr, q * NCH:(q + 1) * NCH],
                    pt[:], mybir.ActivationFunctionType.Relu,
                    bias=b1p_sb[:], scale=1.0)

        # ---- stage 2: per-rh weight matmul -> wT[(rh, k, g)] ----
        # psum->sbuf copies alternate between ACT and the (still idle) DVE
        # so the copy wall halves.
        wT = wpool.tile([124, HQ, 49, G], bf16)
        with tc.tile_pool(name="psum_w", bufs=3,
                          space=bass.MemorySpace.PSUM) as psum_w:
            for rh in range(HQ):
                # [118, 2, 512]: each gh half starts on a psum bank boundary
                pw = psum_w.tile([NCOL, 2, 512], f32)
                for gh in range(2):
                    nc.tensor.matmul(pw[:, gh, 0:392],
                                     t_ext[:, rh, :],
                                     w2e_sb[:, gh * 392:(gh + 1) * 392],
                                     start=True, stop=True)
                # psum col (gh, k*8+g8) -> wT[p, rh, k, gh*8+g8]
                dst = wT[0:NCOL, rh, :, :].rearrange(
                    "p k (gh g8) -> p k gh g8", gh=2)
                src = pw[:, :, 0:392].rearrange(
                    "p gh (k g8) -> p k gh g8", k=49)
                if rh % 2 == 0:
                    nc.scalar.copy(dst, src)
                else:
                    nc.vector.tensor_copy(dst, src)

        # ---- j-shifts: loaded straight from the padded DRAM copy
        # (xt_d[j:j+128]) just-in-time, round-robin over the queues; the
        # DMA engine is idle during the tap window and the PE sheds the
        # 60 shift matmuls (17% of its window work).
        dma_eng = [nc.sync, nc.scalar, nc.gpsimd]
        for j in range(1, KK):
            dma_eng[(j - 1) % 3].dma_start(xTs[j][:], xt_d[j:j + 128])

        # ---- stage 3: involution taps ----
        # DVE computes only the 49 products (one 2x tensor_tensor each);
        # the PE accumulates them into a 7-bank f32 PSUM tile via
        # identity matmuls (psum += I @ prod), removing all adds/merges
        # from the DVE stream.
        prodp = ctx.enter_context(tc.tile_pool(name="prod", bufs=8))
        paccp = ctx.enter_context(tc.tile_pool(
            name="psum_acc", bufs=1, space=bass.MemorySpace.PSUM))
        outp = ctx.enter_context(tc.tile_pool(name="outp", bufs=1))
        pacc = paccp.tile([118, FREE3], f32)

        nd = 0
        for j in range(KK):
          for i in range(KK):
            k = i * KK + j
            src = xTs[j][0:118, i * 256:i * 256 + FREE3]
            wsl = wT[0:118, :, k, :].unsqueeze(2).broadcast_to(
                [118, HQ, Cg, G])
            src4 = src.rearrange("p (rh c) -> p rh c", rh=HQ).rearrange(
                "p rh (cg g) -> p rh cg g", g=G)
            dp = prodp.tile([124, FREE3], bf16, tag="dp")
            dp4 = dp[0:118, :].rearrange(
                "p (rh cg g) -> p rh cg g", rh=HQ, g=G)
            nc.vector.tensor_mul(dp4, src4, wsl)
            for c in range(7):
                nc.tensor.matmul(
                    pacc[:, c * 512:(c + 1) * 512], s_sb[0:118, 0, :],
                    dp[0:118, c * 512:(c + 1) * 512],
                    start=(nd == 0), stop=(nd == 48),
                    skip_group_check=True)
            nd += 1

        # psum -> bf16 sbuf -> 3-way DMA (single shared DMA engine)
        ob = outp.tile([118, FREE3], bf16)
        nc.scalar.copy(ob[:], pacc[:])
        nc.sync.dma_start(out_d[0:40], ob[0:40, :])
        nc.scalar.dma_start(out_d[40:79], ob[40:79, :])
        nc.gpsimd.dma_start(out_d[79:118], ob[79:118, :])

    nc.compile()
    return nc


def _prep_host_inputs(inputs, W1, b1, gamma, beta, mean, var, W2, b2):
    """Fold BN into W1/b1; build per-core transposed layouts."""
    scale = gamma / np.sqrt(var + EPS)
    shift = beta - mean * scale
    W1p = W1 * scale[:, None]
    b1p = (b1 * scale + shift).astype(np.float32).reshape(Cr, 1)
    w1t = np.ascontiguousarray(W1p.T.reshape(2, 128, Cr)).astype(BF16)

    # W2e [65, 784]: row 64 = b2; col n = gh*392 + k*8 + g8 for g = gh*8+g8
    w2full = np.concatenate([W2, b2[:, None]], axis=1)  # [784, 65]
    w2g = w2full.reshape(G, 49, Cr + 1)                 # [g, k, cr+1]
    w2e = w2g.reshape(2, 8, 49, Cr + 1).transpose(3, 0, 2, 1)  # [65,2,49,8]
    w2e = np.ascontiguousarray(w2e.reshape(Cr + 1, 784)).astype(BF16)

    xcms, xts = [], []
    for core in range(NCORES):
        b, hf = core // 2, core % 2
        slab = np.zeros((C, HH + 2 * PAD, PW), np.float32)
        r0 = hf * HH - PAD
        r1 = r0 + HH + 2 * PAD
        v0, v1 = max(r0, 0), min(r1, H)
        slab[:, v0 - r0:v1 - r0, PAD:PAD + W] = inputs[b, :, v0:v1, :]
        slab = slab.astype(BF16)  # [256, 34, 62]

        # x_cm [2, 128, (rh:14, ckpw:118)]: pixel col (rh, ck, pw) is the
        # CENTER value slab[c, PAD + ck*14 + rh, pw'] where the stage-1
        # conv is 1x1 at output pixel (ck*14+rh, pw-PAD): center padded
        # coords = (PAD + ck*14 + rh, pw) with pw = PAD+w... but we keep
        # all 62 cols for ck0 (cols pw=56..61 are junk pixels).
        # column ckpw maps to output pixel (14ck+rh, w=pw): center value
        # is slab[c, PAD + 14ck + rh, PAD + pw]; ck0 cols 56..61 are junk
        xcm = np.zeros((C, HQ, NCOL), BF16)
        xcm[:, :, 0:W] = slab[:, PAD:PAD + HQ, PAD:PAD + W]        # ck0
        xcm[:, :, PW:NCOL] = slab[:, PAD + HQ:PAD + HH, PAD:PAD + W]
        xcms.append(np.ascontiguousarray(xcm.reshape(C, NPIX)
                                         .reshape(2, 128, NPIX)))

        # xT [134, (r:20, cg:16, g:16)]: row 62ck+pw, 10 zero pad rows
        sg = slab.reshape(G, Cg, HH + 2 * PAD, PW)  # [g, cg, 34, 62]
        xt = np.zeros((134, PR, Cg, G), BF16)
        for ck in range(2):
            blk = sg[:, :, ck * HQ:ck * HQ + PR, :]  # [g, cg, 20, 62]
            xt[ck * PW:(ck + 1) * PW] = blk.transpose(3, 2, 1, 0)
        xts.append(np.ascontiguousarray(xt.reshape(134, PR * 256)))

    return xcms, xts, w1t, b1p, w2e


def kernel(inputs, W1, b1, gamma, beta, mean, var, W2, b2):
    global LAST_RESULT
    inputs = np.asarray(inputs, np.float32)
    if "nc" not in _CACHE:
        _CACHE["nc"] = _build_nc()
    nc = _CACHE["nc"]

    xcms, xts, w1t, b1p, w2e = _prep_host_inputs(
        inputs, np.asarray(W1, np.float32), np.asarray(b1, np.float32),
        np.asarray(gamma, np.float32), np.asarray(beta, np.float32),
        np.asarray(mean, np.float32), np.asarray(var, np.float32),
        np.asarray(W2, np.float32), np.asarray(b2, np.float32))

    in_maps = [{"xcm": xcms[core], "xt": xts[core], "w1t": w1t,
                "b1p": b1p, "w2e": w2e} for core in range(NCORES)]
    res = run_bass_kernel_spmd(nc, in_maps, list(range(NCORES)), trace=TRACE)
    LAST_RESULT = res

    out = np.empty((B, C, H, W), np.float32)
    for core in range(NCORES):
        b, hf = core // 2, core % 2
        o = res.results[core]["out"].astype(np.float32)  # [118, 3584] bf16
        o6 = o.reshape(NCOL, HQ, Cg, G)       # [ckpw, rh, cg, g]
        slab = np.empty((C, HH, W), np.float32)
        for ck in range(2):
            cols = o6[ck * PW:ck * PW + W]    # [56, 14, 16, 16]
            # -> [c = g*16+cg, rh, w]
            slab[:, ck * HQ:(ck + 1) * HQ, :] = (
                cols.transpose(3, 2, 1, 0).reshape(C, HQ, W))
        out[b, :, hf * HH:(hf + 1) * HH, :] = slab
    return out


# revision 10
# speedup vs baseline: 2.2732x; 1.0979x over previous
"""Involution (B=4, C=256, H=W=56, K=7, G=16, reduction=4) on 8 trn2 NeuronCores.

v2: pixel-major involution with zero-materialization weight broadcast.

Sharding: 8 shards = (batch b in 0..3) x (h-half hf in 0..1); each core
computes a [256, 28, 56] output slab.

Per-core layout (all bf16 in SBUF):
  - Pixel-major transposed input xT[124, (r:20, cg:16, g:16)]: partition
    p = 62*ck + pw where ck splits the 28 output rows into 2x14 and pw is
    the padded column (0..61); r is the padded row within the ck chunk
    (14+2*3=20); channel c = g*16+cg stored g-innermost so that per-group
    weights broadcast via a stride-0 middle AP dim.
  - Stage 1 (PE+ACT): t_ext[65, (rh:14, ckpw:118)] = [relu(W1p@x+b1p); 1]
    from channel-major x_cm; BN folded into W1p/b1p on host.
  - Stage 2 (PE+ACT): per rh: psum[118, 784] = t_ext[:,rh,:]^T @ W2e
    (cols n = gh*392 + k*8 + g8), one ACT copy -> wT[124,(rh,k,g)] bf16.
  - Stage 3 (DVE+GPSIMD): per tap k=(i,j), ONE op over the whole slab:
      acc += xT[j:j+118, i*256 : i*256+3584] * wT[0:118, :, k, :]-bcast
    The weight read uses AP [(rh,784),(cg,0),(g,1)] - stride-0 broadcast,
    verified to run in DVE 2x mode. ~11 taps run on gpsimd instead.
  - Merge accumulators -> f32, DMA out; host un-permutes.
"""

import numpy as np
import ml_dtypes
from contextlib import ExitStack

import concourse.bass as bass
import concourse.bacc as bacc
import concourse.tile as tile
from concourse import mybir
from concourse.bass_utils import run_bass_kernel_spmd

BF16 = ml_dtypes.bfloat16

B, C, H, W = 4, 256, 56, 56
KK, G, PAD = 7, 16, 3
Cr, Cg = 64, 16
EPS = 1e-5
HH = H // 2              # 28 rows per h-half shard
HQ = HH // 2             # 14 rows per sub-chunk (ck)
PW = W + 2 * PAD         # 62 padded width
PR = HQ + 2 * PAD        # 20 padded rows per sub-chunk
NCOL = PW + W            # 118 = ck0 cols 0..61 + ck1 cols 0..55
NPIX = HQ * NCOL         # 1652 stage-1/2 pixel columns
FREE3 = HQ * 256         # 3584 = stage-3 free size
NCORES = 8

# GPSIMD shares SBUF ports with the DVE: running Pool tensor_tensor ops
# concurrently with DVE taps collapses DVE throughput ~4.3x (measured), so
# stage 3 runs entirely on the DVE.
N_DVE_ACC = 4

_CACHE = {}
TRACE = False
LAST_RESULT = None


def _build_nc():
    nc = bacc.Bacc("TRN2", target_bir_lowering=False, debug=False,
                   num_devices=NCORES)

    f32 = mybir.dt.float32
    bf16 = mybir.dt.bfloat16

    xcm_d = nc.declare_dram_parameter("xcm", [2, 128, NPIX], bf16,
                                      isOutput=False)
    # 134 rows: 124 (ck,pw) rows + 10 zero rows so each shifted load
    # xTs[j] = xt[j : j+128] stays in range for j <= 6
    xt_d = nc.declare_dram_parameter("xt", [134, PR * 256], bf16,
                                     isOutput=False)
    w1t_d = nc.declare_dram_parameter("w1t", [2, 128, Cr], bf16,
                                      isOutput=False)
    b1p_d = nc.declare_dram_parameter("b1p", [Cr, 1], f32, isOutput=False)
    w2e_d = nc.declare_dram_parameter("w2e", [Cr + 1, 784], bf16,
                                      isOutput=False)
    out_d = nc.declare_dram_parameter("out", [118, FREE3], bf16,
                                      isOutput=True)

    with tile.TileContext(nc) as tc, ExitStack() as ctx:
        const = ctx.enter_context(tc.tile_pool(name="const", bufs=1))
        xpool = ctx.enter_context(tc.tile_pool(name="x", bufs=1))
        tpool = ctx.enter_context(tc.tile_pool(name="t", bufs=1))
        wpool = ctx.enter_context(tc.tile_pool(name="w", bufs=1))

        # Only xTs[0] is loaded from DRAM (split across both HWDGE
        # queues); the 6 shifted copies are produced on-chip by the PE
        # (matmul with a shifted identity = partition shift), which keeps
        # the DMA engines quiet during the DVE tap window.
        x_cm = xpool.tile([128, 2, NPIX], bf16)
        nc.sync.dma_start(x_cm[:, 0, :], xcm_d[0])
        nc.scalar.dma_start(x_cm[:, 1, :], xcm_d[1])

        w1t_sb = const.tile([128, 2, Cr], bf16)
        for ch in range(2):
            nc.sync.dma_start(w1t_sb[:, ch, :], w1t_d[ch])
        b1p_sb = const.tile([Cr, 1], f32)
        nc.sync.dma_start(b1p_sb[:], b1p_d[:])
        w2e_sb = const.tile([Cr + 1, 784], bf16)
        nc.scalar.dma_start(w2e_sb[:], w2e_d[:])
        # shift matrices S[kk, j-1, m] = 1 iff kk == m + j, built on-chip
        # (keeps the single shared DMA engine free for the x loads)
        # s_sb[:, 0, :] is the identity (used for PE psum-accumulation
        # of tap products); s_sb[:, j, :] is the j-shift matrix.
        s_sb = const.tile([128, 7, 118], bf16)
        nc.gpsimd.memset(s_sb[:], 1.0)
        nc.gpsimd.affine_select(
            s_sb[:], s_sb[:], pattern=[[-1, 7], [-1, 118]],
            compare_op=mybir.AluOpType.is_equal, fill=0.0, base=0,
            channel_multiplier=1)

        xTs = []
        for j in range(KK):
            xtj = xpool.tile([128, PR * 256], bf16, name=f"xTs{j}")
            xTs.append(xtj)
        nc.sync.dma_start(xTs[0][0:64], xt_d[0:64])
        nc.scalar.dma_start(xTs[0][64:128], xt_d[64:128])

        # ---- stage 1: t_ext = [relu(W1p @ x + b1p); ones] ----
        t_ext = tpool.tile([Cr + 1, HQ, NCOL], bf16)
        nc.vector.memset(t_ext[Cr:Cr + 1, :, :], 1.0)
        t2 = t_ext[:].rearrange("p a b -> p (a b)")
        with tc.tile_pool(name="psum_t", bufs=2,
                          space=bass.MemorySpace.PSUM) as psum_t:
            NCH = 413  # 4 chunks of 413 = 1652 (<= 512 per psum bank)
            for q in range(4):
                pt = psum_t.tile([Cr, NCH], f32)
                for ch in range(2):
                    nc.tensor.matmul(pt[:], w1t_sb[:, ch, :],
                                     x_cm[:, ch, q * NCH:(q + 1) * NCH],
                                     start=(ch == 0), stop=(ch == 1))
                nc.scalar.activation(
                    t2[0:Cr, q * NCH:(q + 1) * NCH],
                    pt[:], mybir.ActivationFunctionType.Relu,
                    bias=b1p_sb[:], scale=1.0)

        # ---- stage 2: per-rh weight matmul -> wT[(rh, k, g)] ----
        # psum->sbuf copies alternate between ACT and the (still idle) DVE
        # so the copy wall halves.
        wT = wpool.tile([124, HQ, 49, G], bf16)
        with tc.tile_pool(name="psum_w", bufs=3,
                          space=bass.MemorySpace.PSUM) as psum_w:
            for rh in range(HQ):
                # [118, 2, 512]: each gh half starts on a psum bank boundary
                pw = psum_w.tile([NCOL, 2, 512], f32)
                for gh in range(2):
                    nc.tensor.matmul(pw[:, gh, 0:392],
                                     t_ext[:, rh, :],
                                     w2e_sb[:, gh * 392:(gh + 1) * 392],
                                     start=True, stop=True)
                # psum col (gh, k*8+g8) -> wT[p, rh, k, gh*8+g8]
                dst = wT[0:NCOL, rh, :, :].rearrange(
                    "p k (gh g8) -> p k gh g8", gh=2)
                src = pw[:, :, 0:392].rearrange(
                    "p gh (k g8) -> p k gh g8", k=49)
                if rh % 2 == 0:
                    nc.scalar.copy(dst, src)
                else:
                    nc.vector.tensor_copy(dst, src)

        # ---- j-shifts: loaded straight from the padded DRAM copy
        # (xt_d[j:j+128]).  All six issue from gpsimd, gated behind a tiny
        # probe read of xTs[0]: the single shared DMA engine then finishes
        # the critical x_cm/xTs[0] loads before starting the shift
        # transfers, which fill the (DMA-idle) tap window just-in-time.
        probe = const.tile([1, 8], bf16)
        nc.gpsimd.tensor_copy(probe[:], xTs[0][0:1, 0:8])
        for j in range(1, KK):
            nc.gpsimd.dma_start(xTs[j][:], xt_d[j:j + 128])

        # ---- stage 3: involution taps ----
        # DVE computes only the 49 products (one 2x tensor_tensor each);
        # the PE accumulates them into a 7-bank f32 PSUM tile via
        # identity matmuls (psum += I @ prod), removing all adds/merges
        # from the DVE stream.
        prodp = ctx.enter_context(tc.tile_pool(name="prod", bufs=8))
        paccp = ctx.enter_context(tc.tile_pool(
            name="psum_acc", bufs=1, space=bass.MemorySpace.PSUM))
        outp = ctx.enter_context(tc.tile_pool(name="outp", bufs=1))
        pacc = paccp.tile([118, FREE3], f32)

        nd = 0
        for j in range(KK):
          for i in range(KK):
            k = i * KK + j
            src = xTs[j][0:118, i * 256:i * 256 + FREE3]
            wsl = wT[0:118, :, k, :].unsqueeze(2).broadcast_to(
                [118, HQ, Cg, G])
            src4 = src.rearrange("p (rh c) -> p rh c", rh=HQ).rearrange(
                "p rh (cg g) -> p rh cg g", g=G)
            dp = prodp.tile([124, FREE3], bf16, tag="dp")
            dp4 = dp[0:118, :].rearrange(
                "p (rh cg g) -> p rh cg g", rh=HQ, g=G)
            nc.vector.tensor_mul(dp4, src4, wsl)
            for c in range(7):
                nc.tensor.matmul(
                    pacc[:, c * 512:(c + 1) * 512], s_sb[0:118, 0, :],
                    dp[0:118, c * 512:(c + 1) * 512],
                    start=(nd == 0), stop=(nd == 48),
                    skip_group_check=True)
            nd += 1

        # psum -> bf16 sbuf -> 3-way DMA (single shared DMA engine)
        ob = outp.tile([118, FREE3], bf16)
        nc.scalar.copy(ob[:], pacc[:])
        nc.sync.dma_start(out_d[0:40], ob[0:40, :])
        nc.scalar.dma_start(out_d[40:79], ob[40:79, :])
        nc.gpsimd.dma_start(out_d[79:118], ob[79:118, :])

    nc.compile()
    return nc


def _prep_host_inputs(inputs, W1, b1, gamma, beta, mean, var, W2, b2):
    """Fold BN into W1/b1; build per-core transposed layouts."""
    scale = gamma / np.sqrt(var + EPS)
    shift = beta - mean * scale
    W1p = W1 * scale[:, None]
    b1p = (b1 * scale + shift).astype(np.float32).reshape(Cr, 1)
    w1t = np.ascontiguousarray(W1p.T.reshape(2, 128, Cr)).astype(BF16)

    # W2e [65, 784]: row 64 = b2; col n = gh*392 + k*8 + g8 for g = gh*8+g8
    w2full = np.concatenate([W2, b2[:, None]], axis=1)  # [784, 65]
    w2g = w2full.reshape(G, 49, Cr + 1)                 # [g, k, cr+1]
    w2e = w2g.reshape(2, 8, 49, Cr + 1).transpose(3, 0, 2, 1)  # [65,2,49,8]
    w2e = np.ascontiguousarray(w2e.reshape(Cr + 1, 784)).astype(BF16)

    xcms, xts = [], []
    for core in range(NCORES):
        b, hf = core // 2, core % 2
        slab = np.zeros((C, HH + 2 * PAD, PW), np.float32)
        r0 = hf * HH - PAD
        r1 = r0 + HH + 2 * PAD
        v0, v1 = max(r0, 0), min(r1, H)
        slab[:, v0 - r0:v1 - r0, PAD:PAD + W] = inputs[b, :, v0:v1, :]
        slab = slab.astype(BF16)  # [256, 34, 62]

        # x_cm [2, 128, (rh:14, ckpw:118)]: pixel col (rh, ck, pw) is the
        # CENTER value slab[c, PAD + ck*14 + rh, pw'] where the stage-1
        # conv is 1x1 at output pixel (ck*14+rh, pw-PAD): center padded
        # coords = (PAD + ck*14 + rh, pw) with pw = PAD+w... but we keep
        # all 62 cols for ck0 (cols pw=56..61 are junk pixels).
        # column ckpw maps to output pixel (14ck+rh, w=pw): center value
        # is slab[c, PAD + 14ck + rh, PAD + pw]; ck0 cols 56..61 are junk
        xcm = np.zeros((C, HQ, NCOL), BF16)
        xcm[:, :, 0:W] = slab[:, PAD:PAD + HQ, PAD:PAD + W]        # ck0
        xcm[:, :, PW:NCOL] = slab[:, PAD + HQ:PAD + HH, PAD:PAD + W]
        xcms.append(np.ascontiguousarray(xcm.reshape(C, NPIX)
                                         .reshape(2, 128, NPIX)))

        # xT [134, (r:20, cg:16, g:16)]: row 62ck+pw, 10 zero pad rows
        sg = slab.reshape(G, Cg, HH + 2 * PAD, PW)  # [g, cg, 34, 62]
        xt = np.zeros((134, PR, Cg, G), BF16)
        for ck in range(2):
            blk = sg[:, :, ck * HQ:ck * HQ + PR, :]  # [g, cg, 20, 62]
            xt[ck * PW:(ck + 1) * PW] = blk.transpose(3, 2, 1, 0)
        xts.append(np.ascontiguousarray(xt.reshape(134, PR * 256)))

    return xcms, xts, w1t, b1p, w2e


def kernel(inputs, W1, b1, gamma, beta, mean, var, W2, b2):
    global LAST_RESULT
    inputs = np.asarray(inputs, np.float32)
    if "nc" not in _CACHE:
        _CACHE["nc"] = _build_nc()
    nc = _CACHE["nc"]

    xcms, xts, w1t, b1p, w2e = _prep_host_inputs(
        inputs, np.asarray(W1, np.float32), np.asarray(b1, np.float32),
        np.asarray(gamma, np.float32), np.asarray(beta, np.float32),
        np.asarray(mean, np.float32), np.asarray(var, np.float32),
        np.asarray(W2, np.float32), np.asarray(b2, np.float32))

    in_maps = [{"xcm": xcms[core], "xt": xts[core], "w1t": w1t,
                "b1p": b1p, "w2e": w2e} for core in range(NCORES)]
    res = run_bass_kernel_spmd(nc, in_maps, list(range(NCORES)), trace=TRACE)
    LAST_RESULT = res

    out = np.empty((B, C, H, W), np.float32)
    for core in range(NCORES):
        b, hf = core // 2, core % 2
        o = res.results[core]["out"].astype(np.float32)  # [118, 3584] bf16
        o6 = o.reshape(NCOL, HQ, Cg, G)       # [ckpw, rh, cg, g]
        slab = np.empty((C, HH, W), np.float32)
        for ck in range(2):
            cols = o6[ck * PW:ck * PW + W]    # [56, 14, 16, 16]
            # -> [c = g*16+cg, rh, w]
            slab[:, ck * HQ:(ck + 1) * HQ, :] = (
                cols.transpose(3, 2, 1, 0).reshape(C, HQ, W))
        out[b, :, hf * HH:(hf + 1) * HH, :] = slab
    return out


# revision 11
# speedup vs baseline: 2.2999x; 1.0117x over previous
"""Involution (B=4, C=256, H=W=56, K=7, G=16, reduction=4) on 8 trn2 NeuronCores.

v2: pixel-major involution with zero-materialization weight broadcast.

Sharding: 8 shards = (batch b in 0..3) x (h-half hf in 0..1); each core
computes a [256, 28, 56] output slab.

Per-core layout (all bf16 in SBUF):
  - Pixel-major transposed input xT[124, (r:20, cg:16, g:16)]: partition
    p = 62*ck + pw where ck splits the 28 output rows into 2x14 and pw is
    the padded column (0..61); r is the padded row within the ck chunk
    (14+2*3=20); channel c = g*16+cg stored g-innermost so that per-group
    weights broadcast via a stride-0 middle AP dim.
  - Stage 1 (PE+ACT): t_ext[65, (rh:14, ckpw:118)] = [relu(W1p@x+b1p); 1]
    from channel-major x_cm; BN folded into W1p/b1p on host.
  - Stage 2 (PE+ACT): per rh: psum[118, 784] = t_ext[:,rh,:]^T @ W2e
    (cols n = gh*392 + k*8 + g8), one ACT copy -> wT[124,(rh,k,g)] bf16.
  - Stage 3 (DVE+GPSIMD): per tap k=(i,j), ONE op over the whole slab:
      acc += xT[j:j+118, i*256 : i*256+3584] * wT[0:118, :, k, :]-bcast
    The weight read uses AP [(rh,784),(cg,0),(g,1)] - stride-0 broadcast,
    verified to run in DVE 2x mode. ~11 taps run on gpsimd instead.
  - Merge accumulators -> f32, DMA out; host un-permutes.
"""

import numpy as np
import ml_dtypes
from contextlib import ExitStack

import concourse.bass as bass
import concourse.bacc as bacc
import concourse.tile as tile
from concourse import mybir
from concourse.bass_utils import run_bass_kernel_spmd

BF16 = ml_dtypes.bfloat16

B, C, H, W = 4, 256, 56, 56
KK, G, PAD = 7, 16, 3
Cr, Cg = 64, 16
EPS = 1e-5
HH = H // 2              # 28 rows per h-half shard
HQ = HH // 2             # 14 rows per sub-chunk (ck)
PW = W + 2 * PAD         # 62 padded width
PR = HQ + 2 * PAD        # 20 padded rows per sub-chunk
NCOL = PW + W            # 118 = ck0 cols 0..61 + ck1 cols 0..55
NPIX = HQ * NCOL         # 1652 stage-1/2 pixel columns
FREE3 = HQ * 256         # 3584 = stage-3 free size
NCORES = 8

# GPSIMD shares SBUF ports with the DVE: running Pool tensor_tensor ops
# concurrently with DVE taps collapses DVE throughput ~4.3x (measured), so
# stage 3 runs entirely on the DVE.
N_DVE_ACC = 4

_CACHE = {}
TRACE = False
LAST_RESULT = None


def _build_nc():
    nc = bacc.Bacc("TRN2", target_bir_lowering=False, debug=False,
                   num_devices=NCORES)

    f32 = mybir.dt.float32
    bf16 = mybir.dt.bfloat16

    xcm_d = nc.declare_dram_parameter("xcm", [2, 128, NPIX], bf16,
                                      isOutput=False)
    # 134 rows: 124 (ck,pw) rows + 10 zero rows so each shifted load
    # xTs[j] = xt[j : j+128] stays in range for j <= 6
    xt_d = nc.declare_dram_parameter("xt", [134, PR * 256], bf16,
                                     isOutput=False)
    w1t_d = nc.declare_dram_parameter("w1t", [2, 128, Cr], bf16,
                                      isOutput=False)
    b1p_d = nc.declare_dram_parameter("b1p", [Cr, 1], f32, isOutput=False)
    w2e_d = nc.declare_dram_parameter("w2e", [Cr + 1, 784], bf16,
                                      isOutput=False)
    out_d = nc.declare_dram_parameter("out", [118, FREE3], bf16,
                                      isOutput=True)

    with tile.TileContext(nc) as tc, ExitStack() as ctx:
        const = ctx.enter_context(tc.tile_pool(name="const", bufs=1))
        xpool = ctx.enter_context(tc.tile_pool(name="x", bufs=1))
        tpool = ctx.enter_context(tc.tile_pool(name="t", bufs=1))
        wpool = ctx.enter_context(tc.tile_pool(name="w", bufs=1))

        # Only xTs[0] is loaded from DRAM (split across both HWDGE
        # queues); the 6 shifted copies are produced on-chip by the PE
        # (matmul with a shifted identity = partition shift), which keeps
        # the DMA engines quiet during the DVE tap window.
        x_cm = xpool.tile([128, 2, NPIX], bf16)
        nc.sync.dma_start(x_cm[:, 0, :], xcm_d[0])
        nc.scalar.dma_start(x_cm[:, 1, :], xcm_d[1])

        w1t_sb = const.tile([128, 2, Cr], bf16)
        for ch in range(2):
            nc.sync.dma_start(w1t_sb[:, ch, :], w1t_d[ch])
        b1p_sb = const.tile([Cr, 1], f32)
        nc.sync.dma_start(b1p_sb[:], b1p_d[:])
        w2e_sb = const.tile([Cr + 1, 784], bf16)
        nc.scalar.dma_start(w2e_sb[:], w2e_d[:])
        # shift matrices S[kk, j-1, m] = 1 iff kk == m + j, built on-chip
        # (keeps the single shared DMA engine free for the x loads)
        # s_sb[:, 0, :] is the identity (used for PE psum-accumulation
        # of tap products); s_sb[:, j, :] is the j-shift matrix.
        s_sb = const.tile([128, 7, 118], bf16)
        nc.gpsimd.memset(s_sb[:], 1.0)
        nc.gpsimd.affine_select(
            s_sb[:], s_sb[:], pattern=[[-1, 7], [-1, 118]],
            compare_op=mybir.AluOpType.is_equal, fill=0.0, base=0,
            channel_multiplier=1)

        xTs = []
        for j in range(KK):
            xtj = xpool.tile([128, PR * 256], bf16, name=f"xTs{j}")
            xTs.append(xtj)
        nc.sync.dma_start(xTs[0][0:64], xt_d[0:64])
        nc.scalar.dma_start(xTs[0][64:128], xt_d[64:128])

        # ---- stage 1: t_ext = [relu(W1p @ x + b1p); ones] ----
        t_ext = tpool.tile([Cr + 1, HQ, NCOL], bf16)
        nc.vector.memset(t_ext[Cr:Cr + 1, :, :], 1.0)
        t2 = t_ext[:].rearrange("p a b -> p (a b)")
        with tc.tile_pool(name="psum_t", bufs=2,
                          space=bass.MemorySpace.PSUM) as psum_t:
            NCH = 413  # 4 chunks of 413 = 1652 (<= 512 per psum bank)
            for q in range(4):
                pt = psum_t.tile([Cr, NCH], f32)
                for ch in range(2):
                    nc.tensor.matmul(pt[:], w1t_sb[:, ch, :],
                                     x_cm[:, ch, q * NCH:(q + 1) * NCH],
                                     start=(ch == 0), stop=(ch == 1))
                nc.scalar.activation(
                    t2[0:Cr, q * NCH:(q + 1) * NCH],
                    pt[:], mybir.ActivationFunctionType.Relu,
                    bias=b1p_sb[:], scale=1.0)

        # ---- stage 2: per-rh weight matmul -> wT[(rh, k, g)] ----
        # psum->sbuf copies alternate between ACT and the (still idle) DVE
        # so the copy wall halves.
        wT = wpool.tile([124, HQ, 49, G], bf16)
        with tc.tile_pool(name="psum_w", bufs=3,
                          space=bass.MemorySpace.PSUM) as psum_w:
            for rh in range(HQ):
                # [118, 2, 512]: each gh half starts on a psum bank boundary
                pw = psum_w.tile([NCOL, 2, 512], f32)
                for gh in range(2):
                    nc.tensor.matmul(pw[:, gh, 0:392],
                                     t_ext[:, rh, :],
                                     w2e_sb[:, gh * 392:(gh + 1) * 392],
                                     start=True, stop=True)
                # psum col (gh, k*8+g8) -> wT[p, rh, k, gh*8+g8]
                dst = wT[0:NCOL, rh, :, :].rearrange(
                    "p k (gh g8) -> p k gh g8", gh=2)
                src = pw[:, :, 0:392].rearrange(
                    "p gh (k g8) -> p k gh g8", k=49)
                if rh % 2 == 0:
                    nc.scalar.copy(dst, src)
                else:
                    nc.vector.tensor_copy(dst, src)

        # ---- j-shifts: loaded straight from the padded DRAM copy
        # (xt_d[j:j+128]).  All six issue from gpsimd, gated behind a tiny
        # probe read of xTs[0]: the single shared DMA engine then finishes
        # the critical x_cm/xTs[0] loads before starting the shift
        # transfers, which fill the (DMA-idle) tap window just-in-time.
        # WAW-gate each shift DMA behind a tiny write that reads xTs[0]:
        # a plain probe can be reordered by the scheduler (no dependency),
        # which let the shift transfers interleave with the critical
        # x_cm/xTs[0] loads on the shared DMA engine.
        for j in range(1, KK):
            nc.gpsimd.tensor_copy(xTs[j][0:1, 0:8], xTs[0][0:1, 0:8])
            nc.gpsimd.dma_start(xTs[j][:], xt_d[j:j + 128])

        # ---- stage 3: involution taps ----
        # DVE computes only the 49 products (one 2x tensor_tensor each);
        # the PE accumulates them into a 7-bank f32 PSUM tile via
        # identity matmuls (psum += I @ prod), removing all adds/merges
        # from the DVE stream.
        prodp = ctx.enter_context(tc.tile_pool(name="prod", bufs=8))
        paccp = ctx.enter_context(tc.tile_pool(
            name="psum_acc", bufs=1, space=bass.MemorySpace.PSUM))
        outp = ctx.enter_context(tc.tile_pool(name="outp", bufs=1))
        pacc = paccp.tile([118, FREE3], f32)

        nd = 0
        for j in range(KK):
          for i in range(KK):
            k = i * KK + j
            src = xTs[j][0:118, i * 256:i * 256 + FREE3]
            wsl = wT[0:118, :, k, :].unsqueeze(2).broadcast_to(
                [118, HQ, Cg, G])
            src4 = src.rearrange("p (rh c) -> p rh c", rh=HQ).rearrange(
                "p rh (cg g) -> p rh cg g", g=G)
            dp = prodp.tile([124, FREE3], bf16, tag="dp")
            dp4 = dp[0:118, :].rearrange(
                "p (rh cg g) -> p rh cg g", rh=HQ, g=G)
            nc.vector.tensor_mul(dp4, src4, wsl)
            for c in range(7):
                nc.tensor.matmul(
                    pacc[:, c * 512:(c + 1) * 512], s_sb[0:118, 0, :],
                    dp[0:118, c * 512:(c + 1) * 512],
                    start=(nd == 0), stop=(nd == 48),
                    skip_group_check=True)
            nd += 1

        # psum -> bf16 sbuf -> 3-way DMA (single shared DMA engine)
        ob = outp.tile([118, FREE3], bf16)
        nc.scalar.copy(ob[:], pacc[:])
        nc.sync.dma_start(out_d[0:40], ob[0:40, :])
        nc.scalar.dma_start(out_d[40:79], ob[40:79, :])
        nc.gpsimd.dma_start(out_d[79:118], ob[79:118, :])

    nc.compile()
    return nc


def _prep_host_inputs(inputs, W1, b1, gamma, beta, mean, var, W2, b2):
    """Fold BN into W1/b1; build per-core transposed layouts."""
    scale = gamma / np.sqrt(var + EPS)
    shift = beta - mean * scale
    W1p = W1 * scale[:, None]
    b1p = (b1 * scale + shift).astype(np.float32).reshape(Cr, 1)
    w1t = np.ascontiguousarray(W1p.T.reshape(2, 128, Cr)).astype(BF16)

    # W2e [65, 784]: row 64 = b2; col n = gh*392 + k*8 + g8 for g = gh*8+g8
    w2full = np.concatenate([W2, b2[:, None]], axis=1)  # [784, 65]
    w2g = w2full.reshape(G, 49, Cr + 1)                 # [g, k, cr+1]
    w2e = w2g.reshape(2, 8, 49, Cr + 1).transpose(3, 0, 2, 1)  # [65,2,49,8]
    w2e = np.ascontiguousarray(w2e.reshape(Cr + 1, 784)).astype(BF16)

    xcms, xts = [], []
    for core in range(NCORES):
        b, hf = core // 2, core % 2
        slab = np.zeros((C, HH + 2 * PAD, PW), np.float32)
        r0 = hf * HH - PAD
        r1 = r0 + HH + 2 * PAD
        v0, v1 = max(r0, 0), min(r1, H)
        slab[:, v0 - r0:v1 - r0, PAD:PAD + W] = inputs[b, :, v0:v1, :]
        slab = slab.astype(BF16)  # [256, 34, 62]

        # x_cm [2, 128, (rh:14, ckpw:118)]: pixel col (rh, ck, pw) is the
        # CENTER value slab[c, PAD + ck*14 + rh, pw'] where the stage-1
        # conv is 1x1 at output pixel (ck*14+rh, pw-PAD): center padded
        # coords = (PAD + ck*14 + rh, pw) with pw = PAD+w... but we keep
        # all 62 cols for ck0 (cols pw=56..61 are junk pixels).
        # column ckpw maps to output pixel (14ck+rh, w=pw): center value
        # is slab[c, PAD + 14ck + rh, PAD + pw]; ck0 cols 56..61 are junk
        xcm = np.zeros((C, HQ, NCOL), BF16)
        xcm[:, :, 0:W] = slab[:, PAD:PAD + HQ, PAD:PAD + W]        # ck0
        xcm[:, :, PW:NCOL] = slab[:, PAD + HQ:PAD + HH, PAD:PAD + W]
        xcms.append(np.ascontiguousarray(xcm.reshape(C, NPIX)
                                         .reshape(2, 128, NPIX)))

        # xT [134, (r:20, cg:16, g:16)]: row 62ck+pw, 10 zero pad rows
        sg = slab.reshape(G, Cg, HH + 2 * PAD, PW)  # [g, cg, 34, 62]
        xt = np.zeros((134, PR, Cg, G), BF16)
        for ck in range(2):
            blk = sg[:, :, ck * HQ:ck * HQ + PR, :]  # [g, cg, 20, 62]
            xt[ck * PW:(ck + 1) * PW] = blk.transpose(3, 2, 1, 0)
        xts.append(np.ascontiguousarray(xt.reshape(134, PR * 256)))

    return xcms, xts, w1t, b1p, w2e


def kernel(inputs, W1, b1, gamma, beta, mean, var, W2, b2):
    global LAST_RESULT
    inputs = np.asarray(inputs, np.float32)
    if "nc" not in _CACHE:
        _CACHE["nc"] = _build_nc()
    nc = _CACHE["nc"]

    xcms, xts, w1t, b1p, w2e = _prep_host_inputs(
        inputs, np.asarray(W1, np.float32), np.asarray(b1, np.float32),
        np.asarray(gamma, np.float32), np.asarray(beta, np.float32),
        np.asarray(mean, np.float32), np.asarray(var, np.float32),
        np.asarray(W2, np.float32), np.asarray(b2, np.float32))

    in_maps = [{"xcm": xcms[core], "xt": xts[core], "w1t": w1t,
                "b1p": b1p, "w2e": w2e} for core in range(NCORES)]
    res = run_bass_kernel_spmd(nc, in_maps, list(range(NCORES)), trace=TRACE)
    LAST_RESULT = res

    out = np.empty((B, C, H, W), np.float32)
    for core in range(NCORES):
        b, hf = core // 2, core % 2
        o = res.results[core]["out"].astype(np.float32)  # [118, 3584] bf16
        o6 = o.reshape(NCOL, HQ, Cg, G)       # [ckpw, rh, cg, g]
        slab = np.empty((C, HH, W), np.float32)
        for ck in range(2):
            cols = o6[ck * PW:ck * PW + W]    # [56, 14, 16, 16]
            # -> [c = g*16+cg, rh, w]
            slab[:, ck * HQ:(ck + 1) * HQ, :] = (
                cols.transpose(3, 2, 1, 0).reshape(C, HQ, W))
        out[b, :, hf * HH:(hf + 1) * HH, :] = slab
    return out
